# revision 17
# baseline (speedup 1.0000x reference)
"""AttentionSubsample Trainium2 kernel (v2).

Full (unsharded) inputs in, full output out. Data-parallel over batch:
32 batches -> 8 NeuronCores x 4 batches each. Weights/biases replicated.

Key structure (vs v1 baseline at 275us):
  - All non-PSUM tensors fp16 (same PE rate as bf16/f32r, 2x DVE modes,
    half the SBUF/DMA).
  - Relative-position bias applied MULTIPLICATIVELY after exp:
    exp(s+b) = exp(s)*exp(b). exp(b) is precomputed on host, resident in
    SBUF (6.5MB, loaded once), applied with one DVE tensor_tensor (2x
    mode) per group -- removes ~114us of f32 PSUM bias adds from DVE and
    the bias DMA stream (26MB) entirely.
  - Softmax sums off the PE: e-chunk tree-accumulate on DVE (fp16, 2x)
    + ONE ones-matmul per head on the accumulated [128,NQ] tile
    (PE 2560 cyc/batch instead of 25600).
  - k-channel BN bias dropped: softmax over n is invariant to per-q
    shifts, and (k+bk).q = k.q + bk.q[q] is exactly such a shift.
  - exp computed with a global -3 logit shift (folded into the ACT bias
    operand); softmax is exactly shift-invariant, and the shift keeps
    exp/sums inside fp16 range (logit max ~9 on this distribution).
  - Software-pipelined emission: o-matmuls of head h-1 are interleaved
    into head h's score groups; projections of batch b+1 are emitted as
    closures consumed throughout batch b's head loop, keeping the PE
    busy while ACT/DVE chew on exp/bias work.
  - Engine placement: Pool (idle in v1) takes the v-projection copies
    and the o-normalize; ACT keeps exp + q-bias + kT copies; DVE has
    the bias-mult, sum tree, reciprocal, hswish.
"""

import sys

if "/opt/trn_rl_repo" not in sys.path:
    sys.path.insert(0, "/opt/trn_rl_repo")

import numpy as np

# --- problem constants (hardcoded, must match the grading reference) ---
B, N, C = 32, 1280, 256
H, KD, D = 8, 64, 128          # heads, key dim, value dim per head
NQ = 320                       # subsampled sequence length
OUT = 384
NCORES = 8
BPC = B // NCORES              # batches per core
EPS = 1e-5
NCH = N // 128                 # 10 n-chunks of 128
GRP = 2                        # chunks per score/exp group
NGR = NCH // GRP               # 5 groups per head
SHIFT = 3.0                    # global logit shift (exact for softmax)

_SUB_IDX = np.concatenate([
    (np.arange(32)[::2][:, None] * 32 + np.arange(32)[::2][None, :]).reshape(-1),
    1024 + (np.arange(16)[::2][:, None] * 16 + np.arange(16)[::2][None, :]).reshape(-1),
])  # [320] subsample row gather


def _prep(inputs):
    """Host-side: fold BN into weights, reorder channels, shard over cores."""
    f32 = np.float32
    f16 = np.float16
    x = np.asarray(inputs["x"], f32)
    g_kv, b_kv = np.asarray(inputs["g_kv"], f32), np.asarray(inputs["b_kv"], f32)
    rm_kv, rv_kv = np.asarray(inputs["rm_kv"], f32), np.asarray(inputs["rv_kv"], f32)
    g_q, b_q = np.asarray(inputs["g_q"], f32), np.asarray(inputs["b_q"], f32)
    rm_q, rv_q = np.asarray(inputs["rm_q"], f32), np.asarray(inputs["rv_q"], f32)
    g_p, b_p = np.asarray(inputs["g_p"], f32), np.asarray(inputs["b_p"], f32)
    rm_p, rv_p = np.asarray(inputs["rm_p"], f32), np.asarray(inputs["rv_p"], f32)
    W_kv = np.asarray(inputs["W_kv"], f32)
    W_q = np.asarray(inputs["W_q"], f32)
    W_p = np.asarray(inputs["W_p"], f32)
    attn_bias = np.asarray(inputs["attn_bias"], f32)
    bias_idxs = np.asarray(inputs["bias_idxs"])

    s_kv = g_kv / np.sqrt(rv_kv + EPS)
    Wkv_f = W_kv * s_kv[:, None]
    bkv_f = b_kv - rm_kv * s_kv
    kidx = np.concatenate([np.arange(h * 192, h * 192 + KD) for h in range(H)])
    vidx = np.concatenate([np.arange(h * 192 + KD, (h + 1) * 192) for h in range(H)])
    # k-channel bias dropped: softmax-invariant (see module docstring)
    wkt = np.ascontiguousarray(Wkv_f[kidx].T).reshape(2, 128, 512).astype(f16)
    wvt = np.ascontiguousarray(Wkv_f[vidx].T).reshape(2, 128, 1024).astype(f16)
    bvd = np.ascontiguousarray(bkv_f[vidx].reshape(8, 128).T)          # [128, H]

    scale = KD ** -0.5
    s_q = g_q / np.sqrt(rv_q + EPS)
    wqt = np.ascontiguousarray((W_q * (s_q * scale)[:, None]).T).reshape(
        2, 128, 512).astype(f16)
    bq = np.ascontiguousarray(((b_q - rm_q * s_q) * scale).reshape(4, 128).T)

    s_p = g_p / np.sqrt(rv_p + EPS)
    wpt = np.ascontiguousarray((W_p * s_p[:, None]).T / 6.0).reshape(
        8, 128, OUT).astype(f16)
    bp = np.ascontiguousarray(np.broadcast_to(b_p - rm_p * s_p, (128, OUT)))

    biasT = attn_bias[:, bias_idxs].transpose(0, 2, 1)                 # [H, N, NQ]
    eb = np.exp(biasT).reshape(H, NCH, 128, NQ).astype(f16)

    xs = x[:, _SUB_IDX, :]                                             # [B, NQ, C]
    in_maps = []
    for i in range(NCORES):
        sl = slice(i * BPC, (i + 1) * BPC)
        xt = np.ascontiguousarray(
            x[sl].transpose(0, 2, 1).reshape(BPC, 2, 128, N).astype(f16))
        xst = np.ascontiguousarray(
            xs[sl].transpose(0, 2, 1).reshape(BPC, 2, 128, NQ).astype(f16))
        in_maps.append({
            "xt": xt, "xst": xst,
            "wkt": wkt, "wvt": wvt, "wqt": wqt, "wpt": wpt,
            "bq": bq, "bv": bvd, "bp": bp,
            "eb": eb, "ones": np.ones((128, 128), f16),
        })
    return in_maps


def _body(tc, a, out_ap):
    import concourse.bass as bass  # noqa: F401
    import concourse.mybir as mybir
    from contextlib import ExitStack

    nc = tc.nc
    f32 = mybir.dt.float32
    f16 = mybir.dt.float16
    AF = mybir.ActivationFunctionType
    ALU = mybir.AluOpType

    with ExitStack() as ctx:
        ctx.enter_context(
            nc.allow_low_precision(reason="fp16 attention path is deliberate; "
                                   "verified against fp32 reference")
        )
        singles = ctx.enter_context(tc.tile_pool(name="singles", bufs=1))
        shiftc = singles.tile([128, 1], f32)
        nc.gpsimd.memset(shiftc, -SHIFT)
        bqs = singles.tile([128, 4], f32)
        nc.sync.dma_start(bqs, a["bq"])
        ones = singles.tile([128, 128], f16)
        nc.sync.dma_start(ones, a["ones"])
        wk = singles.tile([128, 2, 512], f16)
        nc.sync.dma_start(wk, a["wkt"].rearrange("c p j -> p c j"))
        wq = singles.tile([128, 2, 512], f16)
        nc.sync.dma_start(wq, a["wqt"].rearrange("c p j -> p c j"))
        wv = singles.tile([128, 2, 1024], f16)
        bvs = singles.tile([128, H], f32)
        wp = singles.tile([128, 8, OUT], f16)
        bps = singles.tile([128, OUT], f32)
        ebt = singles.tile([128, H, NCH, NQ], f16)

        xt_p = ctx.enter_context(tc.tile_pool(name="xt", bufs=2))
        xst_p = ctx.enter_context(tc.tile_pool(name="xst", bufs=2))
        kt_p = ctx.enter_context(tc.tile_pool(name="kt", bufs=2))
        v_p = ctx.enter_context(tc.tile_pool(name="v", bufs=2))
        qt_p = ctx.enter_context(tc.tile_pool(name="qt", bufs=2))
        e_p = ctx.enter_context(tc.tile_pool(name="e", bufs=12))
        acc_p = ctx.enter_context(tc.tile_pool(name="acc", bufs=2))
        sacc_p = ctx.enter_context(tc.tile_pool(name="sacc", bufs=2))
        rc_p = ctx.enter_context(tc.tile_pool(name="rc", bufs=2))
        ot_p = ctx.enter_context(tc.tile_pool(name="ot", bufs=2))
        hs_p = ctx.enter_context(tc.tile_pool(name="hs", bufs=2))
        ob_p = ctx.enter_context(tc.tile_pool(name="ob", bufs=2))
        ps_work = ctx.enter_context(tc.tile_pool(name="ps_work", bufs=2, space="PSUM"))
        ps_o = ctx.enter_context(tc.tile_pool(name="ps_o", bufs=2, space="PSUM"))
        ps_sum = ctx.enter_context(tc.tile_pool(name="ps_sum", bufs=2, space="PSUM"))

        _wt_n = [0]

        def work_tile():
            _wt_n[0] += 1
            return ps_work.tile([128, GRP, 512], f32, tag="w", name=f"wt{_wt_n[0]}")

        def emit_eb_dma():
            for h in range(H):
                nc.sync.dma_start(
                    ebt[:, h, :, :],
                    a["eb"][h].rearrange("c p q -> p c q"),
                )

        # --- projection emission for one batch, as a list of closures ---
        def proj_units(b, xt, xst, kt, vt, qt, first_batch=False):
            units = []

            def kt_unit(pr, half):
                # half 0: n 0..1024 (two 512 psum banks, ONE merged copy);
                # half 1: n 1024..1280 (256 cols)
                def run():
                    wt = work_tile()
                    if half == 0:
                        for j in range(2):
                            for cc in range(2):
                                nc.tensor.matmul(
                                    wt[:, j, :],
                                    lhsT=wk[:, cc, pr * 128:(pr + 1) * 128],
                                    rhs=xt[:, cc, j * 512:(j + 1) * 512],
                                    start=(cc == 0), stop=(cc == 1),
                                )
                        dst = kt[:, pr, 0:1024].rearrange("p (j n) -> p j n", j=2)
                        src = wt
                    else:
                        for cc in range(2):
                            nc.tensor.matmul(
                                wt[:, 0, :256],
                                lhsT=wk[:, cc, pr * 128:(pr + 1) * 128],
                                rhs=xt[:, cc, 1024:N],
                                start=(cc == 0), stop=(cc == 1),
                            )
                        dst = kt[:, pr, 1024:N]
                        src = wt[:, 0, :256]
                    if pr % 2 == 0:
                        nc.scalar.copy(dst, src)
                    else:
                        nc.vector.tensor_copy(dst, src)
                return run

            def v_unit(cn):
                # both 512-halves of a v n-chunk in one work tile + ONE copy
                def run():
                    wt = work_tile()
                    for hf in range(2):
                        for cc in range(2):
                            nc.tensor.matmul(
                                wt[:, hf, :],
                                lhsT=xt[:, cc, cn * 128:(cn + 1) * 128],
                                rhs=wv[:, cc, hf * 512:(hf + 1) * 512],
                                start=(cc == 0), stop=(cc == 1),
                            )
                    dst = vt[:, cn, :].rearrange("p (j n) -> p j n", j=2)
                    if cn % 3 == 0:
                        nc.scalar.copy(dst, wt)
                    else:
                        nc.vector.tensor_copy(dst, wt)
                return run

            def q_unit(pr):
                def run():
                    ps = work_tile()[:, 0, :]
                    for cc in range(2):
                        nc.tensor.matmul(
                            ps[:, :NQ],
                            lhsT=wq[:, cc, pr * 128:(pr + 1) * 128],
                            rhs=xst[:, cc, :],
                            start=(cc == 0), stop=(cc == 1),
                        )
                    nc.scalar.activation(
                        qt[:, pr, :], ps[:, :NQ], AF.Identity,
                        bias=bqs[:, pr:pr + 1],
                    )
                return run

            for pr in range(4):
                for half in range(2):
                    units.append(kt_unit(pr, half))
            for cn in range(NCH):
                units.append(v_unit(cn))
            for pr in range(4):
                units.append(q_unit(pr))
            return units

        # --- per-batch state for the software pipeline ---
        def head_attention(b, kt, vt, qt, ot, units_next):
            """Head loop with one-head o-matmul lag + proj interleaving."""
            nunits = len(units_next)
            consumed = [0]

            def consume(k):
                while consumed[0] < min(nunits, k):
                    units_next[consumed[0]]()
                    consumed[0] += 1

            e_tiles = [[None] * NGR for _ in range(H)]
            po_tiles = [None] * H
            rc_tiles = [None] * H

            def emit_o(h, g):
                po = po_tiles[h]
                for j in range(GRP):
                    c = GRP * g + j
                    nc.tensor.matmul(
                        po,
                        lhsT=vt[:, c, h * 128:(h + 1) * 128],
                        rhs=e_tiles[h][g][:, j, :],
                        start=(c == 0), stop=(c == NCH - 1),
                    )

            def emit_norm(h):
                # o = po/sums + bv ; hswish pairs after odd heads
                oh = ot[:, h, :]
                nc.vector.tensor_tensor(oh, po_tiles[h], rc_tiles[h], ALU.mult)
                nc.vector.tensor_scalar_add(oh, oh, bvs[:, h:h + 1])

            def emit_hswish(h0, t):
                # t = o*relu6(o+3), 1/6 folded into wp; two heads at once
                o2 = ot[:, h0:h0 + 2, :]
                t2 = t[:, h0:h0 + 2, :]
                nc.vector.tensor_scalar(t2, o2, 3.0, 6.0, ALU.add, ALU.min)
                nc.vector.tensor_scalar(t2, t2, 0.0, None, ALU.max)
                nc.vector.tensor_tensor(t2, t2, o2, ALU.mult)

            # hswish output doubles as out-proj lhsT for a batch PAIR:
            # q merged to 640 = 5x128 exact (vs 2.5x128 per batch)
            if b % 2 == 0:
                t_pair = hs_p.tile([128, H, 2, NQ], f16, tag="t", name=f"t_{b}")
                ts[b] = t_pair
            else:
                t_pair = ts[b - 1]
            t = t_pair[:, :, b % 2, :]

            for h in range(H):
                pr, p0 = h // 2, 64 * (h % 2)
                po_tiles[h] = ps_o.tile([128, NQ], f32, tag="po", name=f"po{b}_{h}")
                acc = acc_p.tile([128, GRP, NQ], f16, tag="acc", name=f"acc{b}_{h}")
                for g in range(NGR):
                    sg = work_tile()
                    for j in range(GRP):
                        c = GRP * g + j
                        nc.tensor.matmul(
                            sg[:, j, :NQ],
                            lhsT=kt[p0:p0 + 64, pr, c * 128:(c + 1) * 128],
                            rhs=qt[p0:p0 + 64, pr, :],
                            start=True, stop=True,
                        )
                    e = e_p.tile([128, GRP, NQ], f16, tag="e", name=f"e{b}_{h}_{g}")
                    e_tiles[h][g] = e
                    nc.scalar.activation(e, sg[:, :, :NQ], AF.Exp, bias=shiftc[:, 0:1])
                    # bias-mult is SBUF-only: mostly Pool (otherwise idle),
                    # ~1/4 on DVE
                    idx = h * NGR + g
                    eng = nc.vector if idx % 4 == 0 else nc.gpsimd
                    eng.tensor_tensor(
                        e, e, ebt[:, h, GRP * g:GRP * (g + 1), :], ALU.mult)
                    # sums tree: accumulate group pairs on DVE
                    if g == 0:
                        pass  # acc starts at g=1 with e0+e1
                    elif g == 1:
                        nc.vector.tensor_tensor(acc, e_tiles[h][0], e, ALU.add)
                    else:
                        nc.vector.tensor_tensor(acc, acc, e, ALU.add)
                    # one-head-lagged o accumulation keeps PE fed during exp
                    if h > 0:
                        emit_o(h - 1, g)
                    else:
                        consume(3 * (g + 1))
                # fold [128,2,NQ] -> [128,NQ], then one ones-matmul for sums
                sacc = sacc_p.tile([128, NQ], f16, tag="sacc", name=f"sacc{b}_{h}")
                nc.vector.tensor_tensor(sacc, acc[:, 0, :], acc[:, 1, :], ALU.add)
                psm = ps_sum.tile([128, NQ], f32, tag="psm", name=f"psm{b}_{h}")
                nc.tensor.matmul(psm, lhsT=ones, rhs=sacc, start=True, stop=True)
                rc_tiles[h] = rc_p.tile([128, NQ], f16, tag="rc", name=f"rc{b}_{h}")
                nc.vector.reciprocal(rc_tiles[h], psm)
                if h > 0:
                    emit_norm(h - 1)
                    if h % 2 == 0:
                        emit_hswish(h - 2, t)
                consume((nunits * (h + 1)) // H)
            # drain last head
            for g in range(NGR):
                emit_o(H - 1, g)
            emit_norm(H - 1)
            emit_hswish(H - 2, t)
            consume(nunits)

        # --- main loop over batches ---
        kts, vts, qts, ts = {}, {}, {}, {}

        def alloc_and_dma(b):
            xt = xt_p.tile([128, 2, N], f16)
            for ns in range(3):
                n0 = ns * 512
                nsz = min(512, N - n0)
                nc.sync.dma_start(
                    xt[:, :, n0:n0 + nsz],
                    a["xt"][b, :, :, n0:n0 + nsz].rearrange("c p n -> p c n"),
                )
            xst = xst_p.tile([128, 2, NQ], f16)
            nc.sync.dma_start(xst, a["xst"][b].rearrange("c p n -> p c n"))
            kts[b] = kt_p.tile([128, 4, N], f16, tag="kt", name=f"kt{b}")
            vts[b] = v_p.tile([128, NCH, 1024], f16, tag="vt", name=f"vt{b}")
            qts[b] = qt_p.tile([128, 4, NQ], f16, tag="qt", name=f"qt{b}")
            return xt, xst

        def emit_outproj(b):
            # output projection for batch pair (b-1, b), q merged to 640
            out_flat = out_ap.rearrange("b q o -> (b q) o")
            t_pair = ts[b - 1]
            for qc in range(5):
                r0 = (b - 1) * NQ + qc * 128
                ps = work_tile()[:, 0, :]
                for dc in range(8):
                    nc.tensor.matmul(
                        ps[:, :OUT],
                        lhsT=t_pair[:, dc, :, :].rearrange(
                            "p bb q -> p (bb q)")[:, qc * 128:(qc + 1) * 128],
                        rhs=wp[:, dc, :],
                        start=(dc == 0), stop=(dc == 7),
                    )
                ob = ob_p.tile([128, OUT], f32)
                nc.vector.tensor_tensor(ob, ps[:, :OUT], bps, ALU.add)
                nc.sync.dma_start(out_flat[r0:r0 + 128, :], ob)

        xts = {}
        xts[0] = alloc_and_dma(0)
        # weights needed later in batch 0: emit DMA behind the first x tiles
        nc.sync.dma_start(wv, a["wvt"].rearrange("c p j -> p c j"))
        nc.sync.dma_start(bvs, a["bv"])
        emit_eb_dma()
        nc.sync.dma_start(wp, a["wpt"].rearrange("c p j -> p c j"))
        nc.sync.dma_start(bps, a["bp"])

        units0 = proj_units(0, *xts[0], kts[0], vts[0], qts[0], first_batch=True)
        for u in units0:
            u()
        for b in range(BPC):
            if b + 1 < BPC:
                xts[b + 1] = alloc_and_dma(b + 1)
                units_next = proj_units(
                    b + 1, *xts[b + 1], kts[b + 1], vts[b + 1], qts[b + 1])
            else:
                units_next = []
            head_attention(b, kts[b], vts[b], qts[b],
                           ot_p.tile([128, H, NQ], f16, tag="ot", name=f"ot{b}"),
                           units_next)
            if b % 2 == 1:
                emit_outproj(b)


def build():
    import concourse.mybir as mybir
    import concourse.tile as tile
    from concourse import bacc

    nc = bacc.Bacc("TRN2", target_bir_lowering=False, debug=False)
    f32, f16 = mybir.dt.float32, mybir.dt.float16
    a = {}

    def din(name, shape, dt=f32):
        a[name] = nc.dram_tensor(name, shape, dt, kind="ExternalInput").ap()

    din("xt", [BPC, 2, 128, N], f16)
    din("xst", [BPC, 2, 128, NQ], f16)
    din("wkt", [2, 128, 512], f16)
    din("wvt", [2, 128, 1024], f16)
    din("wqt", [2, 128, 512], f16)
    din("wpt", [8, 128, OUT], f16)
    din("bq", [128, 4])
    din("bv", [128, H])
    din("bp", [128, OUT])
    din("eb", [H, NCH, 128, NQ], f16)
    din("ones", [128, 128], f16)
    out_ap = nc.dram_tensor("out", [BPC, NQ, OUT], f32, kind="ExternalOutput").ap()

    with tile.TileContext(nc) as tc:
        _body(tc, a, out_ap)
    nc.compile()
    return nc


_NC_CACHE = None


def _get_nc():
    global _NC_CACHE
    if _NC_CACHE is None:
        _NC_CACHE = build()
    return _NC_CACHE


def kernel(**inputs):
    from concourse.bass_utils import run_bass_kernel_spmd

    in_maps = _prep(inputs)
    nc = _get_nc()
    res = run_bass_kernel_spmd(nc, in_maps, list(range(NCORES)))
    out = np.concatenate([res.results[i]["out"] for i in range(NCORES)], axis=0)
    return np.ascontiguousarray(out, dtype=np.float32)


if __name__ == "__main__":
    print("smoke: building bass module...")
    nc = build()
    print("built ok:", sum(len(bb.instructions) for bb in nc.m.functions[0].blocks),
          "instructions")


# revision 18
# speedup vs baseline: 1.0083x; 1.0083x over previous
"""AttentionSubsample Trainium2 kernel (v2).

Full (unsharded) inputs in, full output out. Data-parallel over batch:
32 batches -> 8 NeuronCores x 4 batches each. Weights/biases replicated.

Key structure (vs v1 baseline at 275us):
  - All non-PSUM tensors fp16 (same PE rate as bf16/f32r, 2x DVE modes,
    half the SBUF/DMA).
  - Relative-position bias applied MULTIPLICATIVELY after exp:
    exp(s+b) = exp(s)*exp(b). exp(b) is precomputed on host, resident in
    SBUF (6.5MB, loaded once), applied with one DVE tensor_tensor (2x
    mode) per group -- removes ~114us of f32 PSUM bias adds from DVE and
    the bias DMA stream (26MB) entirely.
  - Softmax sums off the PE: e-chunk tree-accumulate on DVE (fp16, 2x)
    + ONE ones-matmul per head on the accumulated [128,NQ] tile
    (PE 2560 cyc/batch instead of 25600).
  - k-channel BN bias dropped: softmax over n is invariant to per-q
    shifts, and (k+bk).q = k.q + bk.q[q] is exactly such a shift.
  - exp computed with a global -3 logit shift (folded into the ACT bias
    operand); softmax is exactly shift-invariant, and the shift keeps
    exp/sums inside fp16 range (logit max ~9 on this distribution).
  - Software-pipelined emission: o-matmuls of head h-1 are interleaved
    into head h's score groups; projections of batch b+1 are emitted as
    closures consumed throughout batch b's head loop, keeping the PE
    busy while ACT/DVE chew on exp/bias work.
  - Engine placement: Pool (idle in v1) takes the v-projection copies
    and the o-normalize; ACT keeps exp + q-bias + kT copies; DVE has
    the bias-mult, sum tree, reciprocal, hswish.
"""

import sys

if "/opt/trn_rl_repo" not in sys.path:
    sys.path.insert(0, "/opt/trn_rl_repo")

import numpy as np

# --- problem constants (hardcoded, must match the grading reference) ---
B, N, C = 32, 1280, 256
H, KD, D = 8, 64, 128          # heads, key dim, value dim per head
NQ = 320                       # subsampled sequence length
OUT = 384
NCORES = 8
BPC = B // NCORES              # batches per core
EPS = 1e-5
NCH = N // 128                 # 10 n-chunks of 128
GRP = 2                        # chunks per score/exp group
NGR = NCH // GRP               # 5 groups per head
SHIFT = 3.0                    # global logit shift (exact for softmax)

_SUB_IDX = np.concatenate([
    (np.arange(32)[::2][:, None] * 32 + np.arange(32)[::2][None, :]).reshape(-1),
    1024 + (np.arange(16)[::2][:, None] * 16 + np.arange(16)[::2][None, :]).reshape(-1),
])  # [320] subsample row gather


def _prep(inputs):
    """Host-side: fold BN into weights, reorder channels, shard over cores."""
    f32 = np.float32
    f16 = np.float16
    x = np.asarray(inputs["x"], f32)
    g_kv, b_kv = np.asarray(inputs["g_kv"], f32), np.asarray(inputs["b_kv"], f32)
    rm_kv, rv_kv = np.asarray(inputs["rm_kv"], f32), np.asarray(inputs["rv_kv"], f32)
    g_q, b_q = np.asarray(inputs["g_q"], f32), np.asarray(inputs["b_q"], f32)
    rm_q, rv_q = np.asarray(inputs["rm_q"], f32), np.asarray(inputs["rv_q"], f32)
    g_p, b_p = np.asarray(inputs["g_p"], f32), np.asarray(inputs["b_p"], f32)
    rm_p, rv_p = np.asarray(inputs["rm_p"], f32), np.asarray(inputs["rv_p"], f32)
    W_kv = np.asarray(inputs["W_kv"], f32)
    W_q = np.asarray(inputs["W_q"], f32)
    W_p = np.asarray(inputs["W_p"], f32)
    attn_bias = np.asarray(inputs["attn_bias"], f32)
    bias_idxs = np.asarray(inputs["bias_idxs"])

    s_kv = g_kv / np.sqrt(rv_kv + EPS)
    Wkv_f = W_kv * s_kv[:, None]
    bkv_f = b_kv - rm_kv * s_kv
    kidx = np.concatenate([np.arange(h * 192, h * 192 + KD) for h in range(H)])
    vidx = np.concatenate([np.arange(h * 192 + KD, (h + 1) * 192) for h in range(H)])
    # k-channel bias dropped: softmax-invariant (see module docstring)
    wkt = np.ascontiguousarray(Wkv_f[kidx].T).reshape(2, 128, 512).astype(f16)
    wvt = np.ascontiguousarray(Wkv_f[vidx].T).reshape(2, 128, 1024).astype(f16)
    bvd = np.ascontiguousarray(bkv_f[vidx].reshape(8, 128).T)          # [128, H]

    scale = KD ** -0.5
    s_q = g_q / np.sqrt(rv_q + EPS)
    wqt = np.ascontiguousarray((W_q * (s_q * scale)[:, None]).T).reshape(
        2, 128, 512).astype(f16)
    bq = np.ascontiguousarray(((b_q - rm_q * s_q) * scale).reshape(4, 128).T)

    s_p = g_p / np.sqrt(rv_p + EPS)
    wpt = np.ascontiguousarray((W_p * s_p[:, None]).T / 6.0).reshape(
        8, 128, OUT).astype(f16)
    bp = np.ascontiguousarray(np.broadcast_to(b_p - rm_p * s_p, (128, OUT)))

    biasT = attn_bias[:, bias_idxs].transpose(0, 2, 1)                 # [H, N, NQ]
    eb = np.exp(biasT).reshape(H, NCH, 128, NQ).astype(f16)

    xs = x[:, _SUB_IDX, :]                                             # [B, NQ, C]
    in_maps = []
    for i in range(NCORES):
        sl = slice(i * BPC, (i + 1) * BPC)
        xt = np.ascontiguousarray(
            x[sl].transpose(0, 2, 1).reshape(BPC, 2, 128, N).astype(f16))
        xst = np.ascontiguousarray(
            xs[sl].transpose(0, 2, 1).reshape(BPC, 2, 128, NQ).astype(f16))
        in_maps.append({
            "xt": xt, "xst": xst,
            "wkt": wkt, "wvt": wvt, "wqt": wqt, "wpt": wpt,
            "bq": bq, "bv": bvd, "bp": bp,
            "eb": eb, "ones": np.ones((128, 128), f16),
        })
    return in_maps


def _body(tc, a, out_ap):
    import concourse.bass as bass  # noqa: F401
    import concourse.mybir as mybir
    from contextlib import ExitStack

    nc = tc.nc
    f32 = mybir.dt.float32
    f16 = mybir.dt.float16
    AF = mybir.ActivationFunctionType
    ALU = mybir.AluOpType

    with ExitStack() as ctx:
        ctx.enter_context(
            nc.allow_low_precision(reason="fp16 attention path is deliberate; "
                                   "verified against fp32 reference")
        )
        singles = ctx.enter_context(tc.tile_pool(name="singles", bufs=1))
        shiftc = singles.tile([128, 1], f32)
        nc.gpsimd.memset(shiftc, -SHIFT)
        bqs = singles.tile([128, 4], f32)
        nc.sync.dma_start(bqs, a["bq"])
        ones = singles.tile([128, 128], f16)
        nc.sync.dma_start(ones, a["ones"])
        wk = singles.tile([128, 2, 512], f16)
        nc.sync.dma_start(wk, a["wkt"].rearrange("c p j -> p c j"))
        wq = singles.tile([128, 2, 512], f16)
        nc.sync.dma_start(wq, a["wqt"].rearrange("c p j -> p c j"))
        wv = singles.tile([128, 2, 1024], f16)
        bvs = singles.tile([128, H], f32)
        wp = singles.tile([128, 8, OUT], f16)
        bps = singles.tile([128, OUT], f32)
        ebt = singles.tile([128, H, NCH, NQ], f16)

        xt_p = ctx.enter_context(tc.tile_pool(name="xt", bufs=2))
        xst_p = ctx.enter_context(tc.tile_pool(name="xst", bufs=2))
        kt_p = ctx.enter_context(tc.tile_pool(name="kt", bufs=2))
        v_p = ctx.enter_context(tc.tile_pool(name="v", bufs=2))
        qt_p = ctx.enter_context(tc.tile_pool(name="qt", bufs=2))
        e_p = ctx.enter_context(tc.tile_pool(name="e", bufs=12))
        acc_p = ctx.enter_context(tc.tile_pool(name="acc", bufs=2))
        sacc_p = ctx.enter_context(tc.tile_pool(name="sacc", bufs=2))
        rc_p = ctx.enter_context(tc.tile_pool(name="rc", bufs=2))
        ot_p = ctx.enter_context(tc.tile_pool(name="ot", bufs=2))
        hs_p = ctx.enter_context(tc.tile_pool(name="hs", bufs=2))
        ob_p = ctx.enter_context(tc.tile_pool(name="ob", bufs=2))
        ps_work = ctx.enter_context(tc.tile_pool(name="ps_work", bufs=2, space="PSUM"))
        ps_o = ctx.enter_context(tc.tile_pool(name="ps_o", bufs=2, space="PSUM"))
        ps_sum = ctx.enter_context(tc.tile_pool(name="ps_sum", bufs=2, space="PSUM"))

        _wt_n = [0]

        def work_tile():
            _wt_n[0] += 1
            return ps_work.tile([128, GRP, 512], f32, tag="w", name=f"wt{_wt_n[0]}")

        def emit_eb_dma():
            for h in range(H):
                nc.sync.dma_start(
                    ebt[:, h, :, :],
                    a["eb"][h].rearrange("c p q -> p c q"),
                )

        # --- projection emission for one batch, as a list of closures ---
        def proj_units(b, xt, xst, kt, vt, qt, first_batch=False):
            units = []

            def kt_unit(pr, half):
                # half 0: n 0..1024 (two 512 psum banks, ONE merged copy);
                # half 1: n 1024..1280 (256 cols)
                def run():
                    wt = work_tile()
                    if half == 0:
                        for j in range(2):
                            for cc in range(2):
                                nc.tensor.matmul(
                                    wt[:, j, :],
                                    lhsT=wk[:, cc, pr * 128:(pr + 1) * 128],
                                    rhs=xt[:, cc, j * 512:(j + 1) * 512],
                                    start=(cc == 0), stop=(cc == 1),
                                )
                        dst = kt[:, pr, 0:1024].rearrange("p (j n) -> p j n", j=2)
                        src = wt
                    else:
                        for cc in range(2):
                            nc.tensor.matmul(
                                wt[:, 0, :256],
                                lhsT=wk[:, cc, pr * 128:(pr + 1) * 128],
                                rhs=xt[:, cc, 1024:N],
                                start=(cc == 0), stop=(cc == 1),
                            )
                        dst = kt[:, pr, 1024:N]
                        src = wt[:, 0, :256]
                    if pr % 2 == 0:
                        nc.scalar.copy(dst, src)
                    else:
                        nc.vector.tensor_copy(dst, src)
                return run

            def v_unit(cn):
                # both 512-halves of a v n-chunk in one work tile + ONE copy
                def run():
                    wt = work_tile()
                    for hf in range(2):
                        for cc in range(2):
                            nc.tensor.matmul(
                                wt[:, hf, :],
                                lhsT=xt[:, cc, cn * 128:(cn + 1) * 128],
                                rhs=wv[:, cc, hf * 512:(hf + 1) * 512],
                                start=(cc == 0), stop=(cc == 1),
                            )
                    dst = vt[:, cn, :].rearrange("p (j n) -> p j n", j=2)
                    if cn % 3 == 0:
                        nc.scalar.copy(dst, wt)
                    else:
                        nc.vector.tensor_copy(dst, wt)
                return run

            def q_unit(pr):
                def run():
                    ps = work_tile()[:, 0, :]
                    for cc in range(2):
                        nc.tensor.matmul(
                            ps[:, :NQ],
                            lhsT=wq[:, cc, pr * 128:(pr + 1) * 128],
                            rhs=xst[:, cc, :],
                            start=(cc == 0), stop=(cc == 1),
                        )
                    nc.scalar.activation(
                        qt[:, pr, :], ps[:, :NQ], AF.Identity,
                        bias=bqs[:, pr:pr + 1],
                    )
                return run

            for pr in range(4):
                for half in range(2):
                    units.append(kt_unit(pr, half))
            for cn in range(NCH):
                units.append(v_unit(cn))
            for pr in range(4):
                units.append(q_unit(pr))
            return units

        # --- per-batch state for the software pipeline ---
        def head_attention(b, kt, vt, qt, ot, units_next):
            """Head loop with one-head o-matmul lag + proj interleaving."""
            nunits = len(units_next)
            consumed = [0]

            def consume(k):
                while consumed[0] < min(nunits, k):
                    units_next[consumed[0]]()
                    consumed[0] += 1

            e_tiles = [[None] * NGR for _ in range(H)]
            po_tiles = [None] * H
            rc_tiles = [None] * H

            def emit_o(h, g):
                po = po_tiles[h]
                for j in range(GRP):
                    c = GRP * g + j
                    nc.tensor.matmul(
                        po,
                        lhsT=vt[:, c, h * 128:(h + 1) * 128],
                        rhs=e_tiles[h][g][:, j, :],
                        start=(c == 0), stop=(c == NCH - 1),
                    )

            def emit_norm(h):
                # o = po/sums + bv ; hswish pairs after odd heads
                oh = ot[:, h, :]
                nc.vector.tensor_tensor(oh, po_tiles[h], rc_tiles[h], ALU.mult)
                nc.vector.tensor_scalar_add(oh, oh, bvs[:, h:h + 1])

            def emit_hswish(h0, t):
                # t = o*relu6(o+3), 1/6 folded into wp; two heads at once
                o2 = ot[:, h0:h0 + 2, :]
                t2 = t[:, h0:h0 + 2, :]
                nc.vector.tensor_scalar(t2, o2, 3.0, 6.0, ALU.add, ALU.min)
                nc.vector.tensor_scalar(t2, t2, 0.0, None, ALU.max)
                nc.vector.tensor_tensor(t2, t2, o2, ALU.mult)

            # hswish output doubles as out-proj lhsT for a batch PAIR:
            # q merged to 640 = 5x128 exact (vs 2.5x128 per batch)
            if b % 2 == 0:
                t_pair = hs_p.tile([128, H, 2, NQ], f16, tag="t", name=f"t_{b}")
                ts[b] = t_pair
            else:
                t_pair = ts[b - 1]
            t = t_pair[:, :, b % 2, :]
            sacc_tiles = [None] * H

            def head_tail(h):
                # everything past the sum-tree for head h, emitted one head
                # late so the PE's ones-matmul never waits on the tree chain
                psm = ps_sum.tile([128, NQ], f32, tag="psm", name=f"psm{b}_{h}")
                nc.tensor.matmul(psm, lhsT=ones, rhs=sacc_tiles[h],
                                 start=True, stop=True)
                rc_tiles[h] = rc_p.tile([128, NQ], f16, tag="rc", name=f"rc{b}_{h}")
                nc.vector.reciprocal(rc_tiles[h], psm)
                emit_norm(h)
                if h % 2 == 1:
                    emit_hswish(h - 1, t)

            for h in range(H):
                pr, p0 = h // 2, 64 * (h % 2)
                po_tiles[h] = ps_o.tile([128, NQ], f32, tag="po", name=f"po{b}_{h}")
                acc = acc_p.tile([128, GRP, NQ], f16, tag="acc", name=f"acc{b}_{h}")
                for g in range(NGR):
                    sg = work_tile()
                    for j in range(GRP):
                        c = GRP * g + j
                        nc.tensor.matmul(
                            sg[:, j, :NQ],
                            lhsT=kt[p0:p0 + 64, pr, c * 128:(c + 1) * 128],
                            rhs=qt[p0:p0 + 64, pr, :],
                            start=True, stop=True,
                        )
                    e = e_p.tile([128, GRP, NQ], f16, tag="e", name=f"e{b}_{h}_{g}")
                    e_tiles[h][g] = e
                    nc.scalar.activation(e, sg[:, :, :NQ], AF.Exp, bias=shiftc[:, 0:1])
                    # bias-mult is SBUF-only: mostly Pool (otherwise idle),
                    # ~1/4 on DVE
                    idx = h * NGR + g
                    eng = nc.vector if idx % 4 == 0 else nc.gpsimd
                    eng.tensor_tensor(
                        e, e, ebt[:, h, GRP * g:GRP * (g + 1), :], ALU.mult)
                    # sums tree: accumulate group pairs on DVE
                    if g == 0:
                        pass  # acc starts at g=1 with e0+e1
                    elif g == 1:
                        nc.vector.tensor_tensor(acc, e_tiles[h][0], e, ALU.add)
                    else:
                        nc.vector.tensor_tensor(acc, acc, e, ALU.add)
                    # one-head-lagged o accumulation keeps PE fed during exp
                    if h > 0:
                        emit_o(h - 1, g)
                    else:
                        consume(3 * (g + 1))
                # fold [128,2,NQ] -> [128,NQ]; the ones-matmul happens in
                # head_tail one head later
                sacc_tiles[h] = sacc_p.tile([128, NQ], f16, tag="sacc",
                                            name=f"sacc{b}_{h}")
                nc.vector.tensor_tensor(
                    sacc_tiles[h], acc[:, 0, :], acc[:, 1, :], ALU.add)
                if h > 0:
                    head_tail(h - 1)
                consume((nunits * (h + 1)) // H)
            # drain last head
            for g in range(NGR):
                emit_o(H - 1, g)
            head_tail(H - 1)
            consume(nunits)

        # --- main loop over batches ---
        kts, vts, qts, ts = {}, {}, {}, {}

        def alloc_and_dma(b):
            xt = xt_p.tile([128, 2, N], f16)
            for ns in range(3):
                n0 = ns * 512
                nsz = min(512, N - n0)
                nc.sync.dma_start(
                    xt[:, :, n0:n0 + nsz],
                    a["xt"][b, :, :, n0:n0 + nsz].rearrange("c p n -> p c n"),
                )
            xst = xst_p.tile([128, 2, NQ], f16)
            nc.sync.dma_start(xst, a["xst"][b].rearrange("c p n -> p c n"))
            kts[b] = kt_p.tile([128, 4, N], f16, tag="kt", name=f"kt{b}")
            vts[b] = v_p.tile([128, NCH, 1024], f16, tag="vt", name=f"vt{b}")
            qts[b] = qt_p.tile([128, 4, NQ], f16, tag="qt", name=f"qt{b}")
            return xt, xst

        def emit_outproj(b):
            # output projection for batch pair (b-1, b), q merged to 640
            out_flat = out_ap.rearrange("b q o -> (b q) o")
            t_pair = ts[b - 1]
            for qc in range(5):
                r0 = (b - 1) * NQ + qc * 128
                ps = work_tile()[:, 0, :]
                for dc in range(8):
                    nc.tensor.matmul(
                        ps[:, :OUT],
                        lhsT=t_pair[:, dc, :, :].rearrange(
                            "p bb q -> p (bb q)")[:, qc * 128:(qc + 1) * 128],
                        rhs=wp[:, dc, :],
                        start=(dc == 0), stop=(dc == 7),
                    )
                ob = ob_p.tile([128, OUT], f32)
                nc.vector.tensor_tensor(ob, ps[:, :OUT], bps, ALU.add)
                nc.sync.dma_start(out_flat[r0:r0 + 128, :], ob)

        xts = {}
        xts[0] = alloc_and_dma(0)
        # weights needed later in batch 0: emit DMA behind the first x tiles
        nc.sync.dma_start(wv, a["wvt"].rearrange("c p j -> p c j"))
        nc.sync.dma_start(bvs, a["bv"])
        emit_eb_dma()
        nc.sync.dma_start(wp, a["wpt"].rearrange("c p j -> p c j"))
        nc.sync.dma_start(bps, a["bp"])

        units0 = proj_units(0, *xts[0], kts[0], vts[0], qts[0], first_batch=True)
        for u in units0:
            u()
        for b in range(BPC):
            if b + 1 < BPC:
                xts[b + 1] = alloc_and_dma(b + 1)
                units_next = proj_units(
                    b + 1, *xts[b + 1], kts[b + 1], vts[b + 1], qts[b + 1])
            else:
                units_next = []
            head_attention(b, kts[b], vts[b], qts[b],
                           ot_p.tile([128, H, NQ], f16, tag="ot", name=f"ot{b}"),
                           units_next)
            if b % 2 == 1:
                emit_outproj(b)


def build():
    import concourse.mybir as mybir
    import concourse.tile as tile
    from concourse import bacc

    nc = bacc.Bacc("TRN2", target_bir_lowering=False, debug=False)
    f32, f16 = mybir.dt.float32, mybir.dt.float16
    a = {}

    def din(name, shape, dt=f32):
        a[name] = nc.dram_tensor(name, shape, dt, kind="ExternalInput").ap()

    din("xt", [BPC, 2, 128, N], f16)
    din("xst", [BPC, 2, 128, NQ], f16)
    din("wkt", [2, 128, 512], f16)
    din("wvt", [2, 128, 1024], f16)
    din("wqt", [2, 128, 512], f16)
    din("wpt", [8, 128, OUT], f16)
    din("bq", [128, 4])
    din("bv", [128, H])
    din("bp", [128, OUT])
    din("eb", [H, NCH, 128, NQ], f16)
    din("ones", [128, 128], f16)
    out_ap = nc.dram_tensor("out", [BPC, NQ, OUT], f32, kind="ExternalOutput").ap()

    with tile.TileContext(nc) as tc:
        _body(tc, a, out_ap)
    nc.compile()
    return nc


_NC_CACHE = None


def _get_nc():
    global _NC_CACHE
    if _NC_CACHE is None:
        _NC_CACHE = build()
    return _NC_CACHE


def kernel(**inputs):
    from concourse.bass_utils import run_bass_kernel_spmd

    in_maps = _prep(inputs)
    nc = _get_nc()
    res = run_bass_kernel_spmd(nc, in_maps, list(range(NCORES)))
    out = np.concatenate([res.results[i]["out"] for i in range(NCORES)], axis=0)
    return np.ascontiguousarray(out, dtype=np.float32)


if __name__ == "__main__":
    print("smoke: building bass module...")
    nc = build()
    print("built ok:", sum(len(bb.instructions) for bb in nc.m.functions[0].blocks),
          "instructions")


# revision 41
# speedup vs baseline: 1.2857x; 1.2751x over previous
"""AttentionSubsample Trainium2 kernel (v2).

Full (unsharded) inputs in, full output out. Data-parallel over batch:
32 batches -> 8 NeuronCores x 4 batches each. Weights/biases replicated.

Key structure (vs v1 baseline at 275us):
  - All non-PSUM tensors fp16 (same PE rate as bf16/f32r, 2x DVE modes,
    half the SBUF/DMA).
  - Relative-position bias applied MULTIPLICATIVELY after exp:
    exp(s+b) = exp(s)*exp(b). exp(b) is precomputed on host, resident in
    SBUF (6.5MB, loaded once), applied with one DVE tensor_tensor (2x
    mode) per group -- removes ~114us of f32 PSUM bias adds from DVE and
    the bias DMA stream (26MB) entirely.
  - Softmax sums off the PE: e-chunk tree-accumulate on DVE (fp16, 2x)
    + ONE ones-matmul per head on the accumulated [128,NQ] tile
    (PE 2560 cyc/batch instead of 25600).
  - k-channel BN bias dropped: softmax over n is invariant to per-q
    shifts, and (k+bk).q = k.q + bk.q[q] is exactly such a shift.
  - exp computed with a global -3 logit shift (folded into the ACT bias
    operand); softmax is exactly shift-invariant, and the shift keeps
    exp/sums inside fp16 range (logit max ~9 on this distribution).
  - Software-pipelined emission: o-matmuls of head h-1 are interleaved
    into head h's score groups; projections of batch b+1 are emitted as
    closures consumed throughout batch b's head loop, keeping the PE
    busy while ACT/DVE chew on exp/bias work.
  - Engine placement: Pool (idle in v1) takes the v-projection copies
    and the o-normalize; ACT keeps exp + q-bias + kT copies; DVE has
    the bias-mult, sum tree, reciprocal, hswish.
"""

import sys

if "/opt/trn_rl_repo" not in sys.path:
    sys.path.insert(0, "/opt/trn_rl_repo")

import numpy as np

# --- problem constants (hardcoded, must match the grading reference) ---
B, N, C = 32, 1280, 256
H, KD, D = 8, 64, 128          # heads, key dim, value dim per head
NQ = 320                       # subsampled sequence length
OUT = 384
NCORES = 8
BPC = B // NCORES              # batches per core
EPS = 1e-5
NCH = N // 128                 # 10 n-chunks of 128
GRP = 2                        # chunks per score/exp group
NGR = NCH // GRP               # 5 groups per head
SHIFT = 3.0                    # global logit shift (exact for softmax)

_SUB_IDX = np.concatenate([
    (np.arange(32)[::2][:, None] * 32 + np.arange(32)[::2][None, :]).reshape(-1),
    1024 + (np.arange(16)[::2][:, None] * 16 + np.arange(16)[::2][None, :]).reshape(-1),
])  # [320] subsample row gather


def _prep(inputs):
    """Host-side: fold BN into weights, reorder channels, shard over cores."""
    f32 = np.float32
    f16 = np.float16
    x = np.asarray(inputs["x"], f32)
    g_kv, b_kv = np.asarray(inputs["g_kv"], f32), np.asarray(inputs["b_kv"], f32)
    rm_kv, rv_kv = np.asarray(inputs["rm_kv"], f32), np.asarray(inputs["rv_kv"], f32)
    g_q, b_q = np.asarray(inputs["g_q"], f32), np.asarray(inputs["b_q"], f32)
    rm_q, rv_q = np.asarray(inputs["rm_q"], f32), np.asarray(inputs["rv_q"], f32)
    g_p, b_p = np.asarray(inputs["g_p"], f32), np.asarray(inputs["b_p"], f32)
    rm_p, rv_p = np.asarray(inputs["rm_p"], f32), np.asarray(inputs["rv_p"], f32)
    W_kv = np.asarray(inputs["W_kv"], f32)
    W_q = np.asarray(inputs["W_q"], f32)
    W_p = np.asarray(inputs["W_p"], f32)
    attn_bias = np.asarray(inputs["attn_bias"], f32)
    bias_idxs = np.asarray(inputs["bias_idxs"])

    s_kv = g_kv / np.sqrt(rv_kv + EPS)
    Wkv_f = W_kv * s_kv[:, None]
    bkv_f = b_kv - rm_kv * s_kv
    kidx = np.concatenate([np.arange(h * 192, h * 192 + KD) for h in range(H)])
    vidx = np.concatenate([np.arange(h * 192 + KD, (h + 1) * 192) for h in range(H)])
    # k-channel bias dropped: softmax-invariant (see module docstring)
    wkt = np.ascontiguousarray(Wkv_f[kidx].T).reshape(2, 128, 512).astype(f16)
    wvt = np.ascontiguousarray(Wkv_f[vidx].T).reshape(2, 128, 1024).astype(f16)
    bvd = np.ascontiguousarray(bkv_f[vidx].reshape(8, 128).T)          # [128, H]

    scale = KD ** -0.5
    s_q = g_q / np.sqrt(rv_q + EPS)
    wqt = np.ascontiguousarray((W_q * (s_q * scale)[:, None]).T).reshape(
        2, 128, 512).astype(f16)
    bq = np.ascontiguousarray(((b_q - rm_q * s_q) * scale).reshape(4, 128).T)

    s_p = g_p / np.sqrt(rv_p + EPS)
    wpt = np.ascontiguousarray((W_p * s_p[:, None]).T / 6.0).reshape(
        8, 128, OUT).astype(f16)
    bp = np.ascontiguousarray(np.broadcast_to(b_p - rm_p * s_p, (128, OUT)))

    biasT = attn_bias[:, bias_idxs].transpose(0, 2, 1)                 # [H, N, NQ]
    eb = np.exp(biasT).reshape(H, NCH, 128, NQ).astype(f16)

    xs = x[:, _SUB_IDX, :]                                             # [B, NQ, C]
    in_maps = []
    for i in range(NCORES):
        sl = slice(i * BPC, (i + 1) * BPC)
        xt = np.ascontiguousarray(
            x[sl].transpose(0, 2, 1).reshape(BPC, 2, 128, N).astype(f16))
        xst = np.ascontiguousarray(
            xs[sl].transpose(0, 2, 1).reshape(BPC, 2, 128, NQ).astype(f16))
        in_maps.append({
            "xt": xt, "xst": xst,
            "wkt": wkt, "wvt": wvt, "wqt": wqt, "wpt": wpt,
            "bq": bq, "bv": bvd, "bp": bp,
            "eb": eb, "ones": np.ones((128, 128), f16),
        })
    return in_maps


def _body(tc, a, out_ap):
    import concourse.bass as bass  # noqa: F401
    import concourse.mybir as mybir
    from contextlib import ExitStack

    nc = tc.nc
    f32 = mybir.dt.float32
    f16 = mybir.dt.float16
    AF = mybir.ActivationFunctionType
    ALU = mybir.AluOpType

    with ExitStack() as ctx:
        ctx.enter_context(
            nc.allow_low_precision(reason="fp16 attention path is deliberate; "
                                   "verified against fp32 reference")
        )
        singles = ctx.enter_context(tc.tile_pool(name="singles", bufs=1))
        shiftc = singles.tile([128, 1], f32)
        nc.gpsimd.memset(shiftc, -SHIFT)
        bqs = singles.tile([128, 4], f32)
        nc.sync.dma_start(bqs, a["bq"])
        ones = singles.tile([128, 128], f16)
        nc.sync.dma_start(ones, a["ones"])
        wk = singles.tile([128, 2, 512], f16)
        nc.sync.dma_start(wk, a["wkt"].rearrange("c p j -> p c j"))
        wq = singles.tile([128, 2, 512], f16)
        nc.sync.dma_start(wq, a["wqt"].rearrange("c p j -> p c j"))
        wv = singles.tile([128, 2, 1024], f16)
        bvs = singles.tile([128, H], f32)
        wp = singles.tile([128, 8, OUT], f16)
        bps = singles.tile([128, OUT], f32)
        ebt = singles.tile([128, H, NCH, NQ], f16)

        xt_p = ctx.enter_context(tc.tile_pool(name="xt", bufs=2))
        xst_p = ctx.enter_context(tc.tile_pool(name="xst", bufs=2))
        kt_p = ctx.enter_context(tc.tile_pool(name="kt", bufs=2))
        v_p = ctx.enter_context(tc.tile_pool(name="v", bufs=2))
        qt_p = ctx.enter_context(tc.tile_pool(name="qt", bufs=2))
        e_p = ctx.enter_context(tc.tile_pool(name="e", bufs=20))
        acc_p = ctx.enter_context(tc.tile_pool(name="acc", bufs=3))
        sacc_p = ctx.enter_context(tc.tile_pool(name="sacc", bufs=3))
        rc_p = ctx.enter_context(tc.tile_pool(name="rc", bufs=3))
        ot_p = ctx.enter_context(tc.tile_pool(name="ot", bufs=1))
        hs_p = ctx.enter_context(tc.tile_pool(name="hs", bufs=2))
        ob_p = ctx.enter_context(tc.tile_pool(name="ob", bufs=2))
        # scores pool: consumed ONLY by exp (ACT) so the sc->exp rotation never
        # chains behind copies; unit pool: proj/outproj psums, copy consumers
        ps_work = ctx.enter_context(tc.tile_pool(name="ps_work", bufs=2, space="PSUM"))
        ps_unit = ctx.enter_context(tc.tile_pool(name="ps_unit", bufs=2, space="PSUM"))
        ps_o = ctx.enter_context(tc.tile_pool(name="ps_o", bufs=1, space="PSUM"))
        ps_sum = ctx.enter_context(tc.tile_pool(name="ps_sum", bufs=1, space="PSUM"))

        _wt_n = [0]

        def work_tile():
            _wt_n[0] += 1
            return ps_work.tile([128, GRP, 512], f32, tag="w", name=f"wt{_wt_n[0]}")

        def unit_tile():
            _wt_n[0] += 1
            return ps_unit.tile([128, 512], f32, tag="u", name=f"ut{_wt_n[0]}")

        _cp_n = [0]

        def copy_engine():
            # ~60/40 ACT/DVE keeps the two psum-capable engines level
            _cp_n[0] += 1
            return nc.scalar.copy if _cp_n[0] % 5 < 3 else nc.vector.tensor_copy

        def emit_eb_dma():
            for h in range(H):
                nc.sync.dma_start(
                    ebt[:, h, :, :],
                    a["eb"][h].rearrange("c p q -> p c q"),
                )

        # --- projection emission for one batch, as a list of closures ---
        def proj_units(b, xt, xst, kt, vt, qt, first_batch=False):
            units = []

            def kt_unit(pr, ns):
                def run():
                    n0 = ns * 512
                    nsz = min(512, N - n0)
                    ps = unit_tile()
                    for cc in range(2):
                        nc.tensor.matmul(
                            ps[:, :nsz],
                            lhsT=wk[:, cc, pr * 128:(pr + 1) * 128],
                            rhs=xt[:, cc, n0:n0 + nsz],
                            start=(cc == 0), stop=(cc == 1),
                        )
                    copy_engine()(kt[:, pr, n0:n0 + nsz], ps[:, :nsz])
                return run

            def v_unit(cn, hf):
                def run():
                    ps = unit_tile()
                    for cc in range(2):
                        nc.tensor.matmul(
                            ps,
                            lhsT=xt[:, cc, cn * 128:(cn + 1) * 128],
                            rhs=wv[:, cc, hf * 512:(hf + 1) * 512],
                            start=(cc == 0), stop=(cc == 1),
                        )
                    copy_engine()(vt[:, cn, hf * 512:(hf + 1) * 512], ps)
                return run

            def q_unit(pr):
                def run():
                    ps = unit_tile()
                    for cc in range(2):
                        nc.tensor.matmul(
                            ps[:, :NQ],
                            lhsT=wq[:, cc, pr * 128:(pr + 1) * 128],
                            rhs=xst[:, cc, :],
                            start=(cc == 0), stop=(cc == 1),
                        )
                    nc.scalar.activation(
                        qt[:, pr, :], ps[:, :NQ], AF.Identity,
                        bias=bqs[:, pr:pr + 1],
                    )
                return run

            for pr in range(4):
                for ns in range(3):
                    units.append(kt_unit(pr, ns))
            for cn in range(NCH):
                for hf in range(2):
                    units.append(v_unit(cn, hf))
            for pr in range(4):
                units.append(q_unit(pr))
            return units

        # --- per-batch state for the software pipeline ---
        def head_attention(b, kt, vt, qt, ot, units_next):
            """Head loop with one-head o-matmul lag + proj interleaving."""
            nunits = len(units_next)
            consumed = [0]

            def consume(k):
                while consumed[0] < min(nunits, k):
                    units_next[consumed[0]]()
                    consumed[0] += 1

            e_tiles = [[None] * NGR for _ in range(H)]
            po_tiles = [None] * H
            rc_tiles = [None] * H

            def emit_o(h, g):
                po = po_tiles[h]
                for j in range(GRP):
                    c = GRP * g + j
                    nc.tensor.matmul(
                        po,
                        lhsT=vt[:, c, h * 128:(h + 1) * 128],
                        rhs=e_tiles[h][g][:, j, :],
                        start=(c == 0), stop=(c == NCH - 1),
                    )

            def emit_norm(h):
                # o = po/sums + bv ; hswish pairs after odd heads
                oh = ot[:, h, :]
                nc.vector.tensor_tensor(oh, po_tiles[h], rc_tiles[h], ALU.mult)
                nc.vector.tensor_scalar_add(oh, oh, bvs[:, h:h + 1])

            def emit_hswish(h0, t):
                # t = o*relu6(o+3), 1/6 folded into wp; two heads at once
                o2 = ot[:, h0:h0 + 2, :]
                t2 = t[:, h0:h0 + 2, :]
                nc.vector.tensor_scalar(t2, o2, 3.0, 6.0, ALU.add, ALU.min)
                nc.vector.tensor_scalar(t2, t2, 0.0, None, ALU.max)
                nc.vector.tensor_tensor(t2, t2, o2, ALU.mult)

            # hswish output doubles as out-proj lhsT for a batch PAIR:
            # q merged to 640 = 5x128 exact (vs 2.5x128 per batch)
            if b % 2 == 0:
                t_pair = hs_p.tile([128, H, 2, NQ], f16, tag="t", name=f"t_{b}")
                ts[b] = t_pair
            else:
                t_pair = ts[b - 1]
            t = t_pair[:, :, b % 2, :]
            acc_tiles = [None] * H

            def head_tail(h):
                # everything past the in-loop sum-tree for head h, emitted one
                # head late so no engine waits on the exp->mult->add chain.
                # group 0's chunk sums go straight to PE ones-matmuls (shifts
                # one tree add off the busier DVE).
                acc = acc_tiles[h]
                nc.vector.tensor_tensor(acc, acc, e_tiles[h][NGR - 2], ALU.add)
                nc.vector.tensor_tensor(acc, acc, e_tiles[h][NGR - 1], ALU.add)
                sacc = sacc_p.tile([128, NQ], f16, tag="sacc",
                                   name=f"sacc{b}_{h}")
                nc.vector.tensor_tensor(sacc, acc[:, 0, :], acc[:, 1, :], ALU.add)
                psm = ps_sum.tile([128, NQ], f32, tag="psm", name=f"psm{b}_{h}")
                nc.tensor.matmul(psm, lhsT=ones, rhs=sacc, start=True, stop=True)
                rc_tiles[h] = rc_p.tile([128, NQ], f16, tag="rc", name=f"rc{b}_{h}")
                nc.vector.reciprocal(rc_tiles[h], psm)
                emit_norm(h)
                if h % 2 == 1:
                    emit_hswish(h - 1, t)

            for h in range(H):
                pr, p0 = h // 2, 64 * (h % 2)
                po_tiles[h] = ps_o.tile([128, NQ], f32, tag="po", name=f"po{b}_{h}")
                acc = acc_p.tile([128, GRP, NQ], f16, tag="acc", name=f"acc{b}_{h}")
                acc_tiles[h] = acc
                for g in range(NGR):
                    # READY work first: in-order engines, so anything emitted
                    # after the (dependency-blocked) scores would stall behind
                    # them. o-matmuls of head h-1 and proj units have ancient
                    # deps and fill the PE while exp/mult of g-2..g-1 complete.
                    if h > 0:
                        emit_o(h - 1, g)
                        consume((nunits * (h * NGR + g + 1)) // (H * NGR))
                    else:
                        consume(3 * (g + 1))
                    sg = work_tile()
                    for j in range(GRP):
                        c = GRP * g + j
                        nc.tensor.matmul(
                            sg[:, j, :NQ],
                            lhsT=kt[p0:p0 + 64, pr, c * 128:(c + 1) * 128],
                            rhs=qt[p0:p0 + 64, pr, :],
                            start=True, stop=True,
                        )
                    e = e_p.tile([128, GRP, NQ], f16, tag="e", name=f"e{b}_{h}_{g}")
                    e_tiles[h][g] = e
                    nc.scalar.activation(e, sg[:, :, :NQ], AF.Exp, bias=shiftc[:, 0:1])
                    # bias-mult is SBUF-only: Pool takes the middle groups,
                    # DVE the first/last
                    eng = nc.vector if g in (0, NGR - 1) else nc.gpsimd
                    eng.tensor_tensor(
                        e, e, ebt[:, h, GRP * g:GRP * (g + 1), :], ALU.mult)
                    # sums tree on DVE, two groups STALE so the add never
                    # waits on a Pool mult in flight (e0 is summed by PE
                    # ones-matmuls in head_tail instead)
                    if g == 3:
                        nc.vector.tensor_tensor(
                            acc, e_tiles[h][0], e_tiles[h][1], ALU.add)
                    elif g == 4:
                        nc.vector.tensor_tensor(
                            acc, acc, e_tiles[h][2], ALU.add)
                if h > 0:
                    head_tail(h - 1)
            # drain last head
            for g in range(NGR):
                emit_o(H - 1, g)
            head_tail(H - 1)
            consume(nunits)

        # --- main loop over batches ---
        kts, vts, qts, ts = {}, {}, {}, {}

        def alloc_and_dma(b):
            xt = xt_p.tile([128, 2, N], f16)
            for ns in range(3):
                n0 = ns * 512
                nsz = min(512, N - n0)
                nc.sync.dma_start(
                    xt[:, :, n0:n0 + nsz],
                    a["xt"][b, :, :, n0:n0 + nsz].rearrange("c p n -> p c n"),
                )
            xst = xst_p.tile([128, 2, NQ], f16)
            nc.sync.dma_start(xst, a["xst"][b].rearrange("c p n -> p c n"))
            kts[b] = kt_p.tile([128, 4, N], f16, tag="kt", name=f"kt{b}")
            vts[b] = v_p.tile([128, NCH, 1024], f16, tag="vt", name=f"vt{b}")
            qts[b] = qt_p.tile([128, 4, NQ], f16, tag="qt", name=f"qt{b}")
            return xt, xst

        def outproj_units(b):
            # output projection for batch pair (b-1, b), q merged to 640;
            # returned as closures consumed inside the NEXT batch's head loop
            out_flat = out_ap.rearrange("b q o -> (b q) o")
            t_pair = ts[b - 1]

            def qc_unit(qc):
                def run():
                    r0 = (b - 1) * NQ + qc * 128
                    ps = unit_tile()
                    for dc in range(8):
                        nc.tensor.matmul(
                            ps[:, :OUT],
                            lhsT=t_pair[:, dc, :, :].rearrange(
                                "p bb q -> p (bb q)")[:, qc * 128:(qc + 1) * 128],
                            rhs=wp[:, dc, :],
                            start=(dc == 0), stop=(dc == 7),
                        )
                    ob = ob_p.tile([128, OUT], f32)
                    nc.vector.tensor_tensor(ob, ps[:, :OUT], bps, ALU.add)
                    nc.sync.dma_start(out_flat[r0:r0 + 128, :], ob)
                return run

            return [qc_unit(qc) for qc in range(5)]

        xts = {}
        xts[0] = alloc_and_dma(0)
        # weights needed later in batch 0: emit DMA behind the first x tiles
        nc.sync.dma_start(wv, a["wvt"].rearrange("c p j -> p c j"))
        nc.sync.dma_start(bvs, a["bv"])
        emit_eb_dma()
        nc.sync.dma_start(wp, a["wpt"].rearrange("c p j -> p c j"))
        nc.sync.dma_start(bps, a["bp"])

        units0 = proj_units(0, *xts[0], kts[0], vts[0], qts[0], first_batch=True)
        for u in units0:
            u()
        carry_out = []
        for b in range(BPC):
            if b + 1 < BPC:
                xts[b + 1] = alloc_and_dma(b + 1)
                units_next = proj_units(
                    b + 1, *xts[b + 1], kts[b + 1], vts[b + 1], qts[b + 1])
            else:
                units_next = []
            units = list(units_next)
            # weave last pair's out-proj into the early-middle of this batch
            for i, u in enumerate(carry_out):
                pos = min(len(units), 6 + 7 * i)
                units.insert(pos, u)
            head_attention(b, kts[b], vts[b], qts[b],
                           ot_p.tile([128, H, NQ], f16, tag="ot", name=f"ot{b}"),
                           units)
            carry_out = outproj_units(b) if b % 2 == 1 else []
        for u in carry_out:
            u()


def build():
    import concourse.mybir as mybir
    import concourse.tile as tile
    from concourse import bacc

    nc = bacc.Bacc("TRN2", target_bir_lowering=False, debug=False)
    f32, f16 = mybir.dt.float32, mybir.dt.float16
    a = {}

    def din(name, shape, dt=f32):
        a[name] = nc.dram_tensor(name, shape, dt, kind="ExternalInput").ap()

    din("xt", [BPC, 2, 128, N], f16)
    din("xst", [BPC, 2, 128, NQ], f16)
    din("wkt", [2, 128, 512], f16)
    din("wvt", [2, 128, 1024], f16)
    din("wqt", [2, 128, 512], f16)
    din("wpt", [8, 128, OUT], f16)
    din("bq", [128, 4])
    din("bv", [128, H])
    din("bp", [128, OUT])
    din("eb", [H, NCH, 128, NQ], f16)
    din("ones", [128, 128], f16)
    out_ap = nc.dram_tensor("out", [BPC, NQ, OUT], f32, kind="ExternalOutput").ap()

    with tile.TileContext(nc) as tc:
        _body(tc, a, out_ap)
    nc.compile()
    return nc


_NC_CACHE = None


def _get_nc():
    global _NC_CACHE
    if _NC_CACHE is None:
        _NC_CACHE = build()
    return _NC_CACHE


def kernel(**inputs):
    from concourse.bass_utils import run_bass_kernel_spmd

    in_maps = _prep(inputs)
    nc = _get_nc()
    res = run_bass_kernel_spmd(nc, in_maps, list(range(NCORES)))
    out = np.concatenate([res.results[i]["out"] for i in range(NCORES)], axis=0)
    return np.ascontiguousarray(out, dtype=np.float32)


if __name__ == "__main__":
    print("smoke: building bass module...")
    nc = build()
    print("built ok:", sum(len(bb.instructions) for bb in nc.m.functions[0].blocks),
          "instructions")


# revision 59
# speedup vs baseline: 1.3469x; 1.0476x over previous
"""AttentionSubsample Trainium2 kernel (v2).

Full (unsharded) inputs in, full output out. Data-parallel over batch:
32 batches -> 8 NeuronCores x 4 batches each. Weights/biases replicated.

Key structure (vs v1 baseline at 275us; cost model now ~227us):
  - All non-PSUM tensors fp16 (same PE rate as bf16/f32r, 2x DVE modes,
    half the SBUF/DMA). Also ~7x more accurate than the bf16 v1.
  - Relative-position bias applied MULTIPLICATIVELY after exp:
    exp(s+b) = exp(s)*exp(b). exp(b) is precomputed on host, resident in
    SBUF (6.5MB, loaded once), applied with one tensor_tensor per group
    (mostly on the otherwise-idle Pool engine, DVE for first/last group)
    -- removes ~114us of f32 PSUM bias adds from DVE and the 26MB bias
    DMA stream entirely.
  - Softmax sums off the PE: e-chunk tree-accumulate on DVE (fp16, 2x,
    deps kept >=2 groups stale) + ONE ones-matmul per head on the
    accumulated [128,NQ] tile (PE 2560 cyc/batch instead of 25600).
  - k-channel BN bias dropped: softmax over n is invariant to per-q
    shifts, and (k+bk).q = k.q + bk.q[q] is exactly such a shift.
  - exp computed with a global -3 logit shift (ACT bias operand);
    softmax is exactly shift-invariant, and the shift keeps exp/sums
    inside fp16 range (logit max ~9 on this distribution).
  - PSUM: scores rotate through a dedicated 2-buf pool whose ONLY
    consumer is exp (ACT), so the sc->exp cadence never chains behind
    copies; projection/out-proj psums use a separate 2x1-bank pool
    (copy consumers); po/psm one bank each. Engines are in-order, so
    all ready work (o-matmuls of head h-1, proj units of batch b+1,
    out-proj of the previous pair) is emitted BEFORE each group's
    dependency-blocked scores, and the per-head tail (tree finish,
    ones-matmul, recip, normalize, hswish) is emitted one head late.
  - Batch-0 projections borrow the then-idle scores pool to double the
    warmup psum rotation depth.
"""

import sys

if "/opt/trn_rl_repo" not in sys.path:
    sys.path.insert(0, "/opt/trn_rl_repo")

import numpy as np

# --- problem constants (hardcoded, must match the grading reference) ---
B, N, C = 32, 1280, 256
H, KD, D = 8, 64, 128          # heads, key dim, value dim per head
NQ = 320                       # subsampled sequence length
OUT = 384
NCORES = 8
BPC = B // NCORES              # batches per core
EPS = 1e-5
NCH = N // 128                 # 10 n-chunks of 128
GRP = 2                        # chunks per score/exp group
NGR = NCH // GRP               # 5 groups per head
SHIFT = 3.0                    # global logit shift (exact for softmax)

_SUB_IDX = np.concatenate([
    (np.arange(32)[::2][:, None] * 32 + np.arange(32)[::2][None, :]).reshape(-1),
    1024 + (np.arange(16)[::2][:, None] * 16 + np.arange(16)[::2][None, :]).reshape(-1),
])  # [320] subsample row gather


def _prep(inputs):
    """Host-side: fold BN into weights, reorder channels, shard over cores."""
    f32 = np.float32
    f16 = np.float16
    x = np.asarray(inputs["x"], f32)
    g_kv, b_kv = np.asarray(inputs["g_kv"], f32), np.asarray(inputs["b_kv"], f32)
    rm_kv, rv_kv = np.asarray(inputs["rm_kv"], f32), np.asarray(inputs["rv_kv"], f32)
    g_q, b_q = np.asarray(inputs["g_q"], f32), np.asarray(inputs["b_q"], f32)
    rm_q, rv_q = np.asarray(inputs["rm_q"], f32), np.asarray(inputs["rv_q"], f32)
    g_p, b_p = np.asarray(inputs["g_p"], f32), np.asarray(inputs["b_p"], f32)
    rm_p, rv_p = np.asarray(inputs["rm_p"], f32), np.asarray(inputs["rv_p"], f32)
    W_kv = np.asarray(inputs["W_kv"], f32)
    W_q = np.asarray(inputs["W_q"], f32)
    W_p = np.asarray(inputs["W_p"], f32)
    attn_bias = np.asarray(inputs["attn_bias"], f32)
    bias_idxs = np.asarray(inputs["bias_idxs"])

    s_kv = g_kv / np.sqrt(rv_kv + EPS)
    Wkv_f = W_kv * s_kv[:, None]
    bkv_f = b_kv - rm_kv * s_kv
    kidx = np.concatenate([np.arange(h * 192, h * 192 + KD) for h in range(H)])
    vidx = np.concatenate([np.arange(h * 192 + KD, (h + 1) * 192) for h in range(H)])
    # k-channel bias dropped: softmax-invariant (see module docstring)
    wkt = np.ascontiguousarray(Wkv_f[kidx].T).reshape(2, 128, 512).astype(f16)
    wvt = np.ascontiguousarray(Wkv_f[vidx].T).reshape(2, 128, 1024).astype(f16)
    bvd = np.ascontiguousarray(bkv_f[vidx].reshape(8, 128).T)          # [128, H]

    scale = KD ** -0.5
    s_q = g_q / np.sqrt(rv_q + EPS)
    wqt = np.ascontiguousarray((W_q * (s_q * scale)[:, None]).T).reshape(
        2, 128, 512).astype(f16)
    bq = np.ascontiguousarray(((b_q - rm_q * s_q) * scale).reshape(4, 128).T)

    s_p = g_p / np.sqrt(rv_p + EPS)
    wpt = np.ascontiguousarray((W_p * s_p[:, None]).T / 6.0).reshape(
        8, 128, OUT).astype(f16)
    bp = np.ascontiguousarray(np.broadcast_to(b_p - rm_p * s_p, (128, OUT)))

    biasT = attn_bias[:, bias_idxs].transpose(0, 2, 1)                 # [H, N, NQ]
    eb = np.exp(biasT).reshape(H, NCH, 128, NQ).astype(f16)

    xs = x[:, _SUB_IDX, :]                                             # [B, NQ, C]
    in_maps = []
    for i in range(NCORES):
        sl = slice(i * BPC, (i + 1) * BPC)
        xt = np.ascontiguousarray(
            x[sl].transpose(0, 2, 1).reshape(BPC, 2, 128, N).astype(f16))
        xst = np.ascontiguousarray(
            xs[sl].transpose(0, 2, 1).reshape(BPC, 2, 128, NQ).astype(f16))
        in_maps.append({
            "xt": xt, "xst": xst,
            "wkt": wkt, "wvt": wvt, "wqt": wqt, "wpt": wpt,
            "bq": bq, "bv": bvd, "bp": bp,
            "eb": eb, "ones": np.ones((128, 128), f16),
        })
    return in_maps


def _body(tc, a, out_ap):
    import concourse.bass as bass  # noqa: F401
    import concourse.mybir as mybir
    from contextlib import ExitStack

    nc = tc.nc
    f32 = mybir.dt.float32
    f16 = mybir.dt.float16
    AF = mybir.ActivationFunctionType
    ALU = mybir.AluOpType

    with ExitStack() as ctx:
        ctx.enter_context(
            nc.allow_low_precision(reason="fp16 attention path is deliberate; "
                                   "verified against fp32 reference")
        )
        singles = ctx.enter_context(tc.tile_pool(name="singles", bufs=1))
        shiftc = singles.tile([128, 1], f32)
        nc.gpsimd.memset(shiftc, -SHIFT)
        bqs = singles.tile([128, 4], f32)
        nc.sync.dma_start(bqs, a["bq"])
        ones = singles.tile([128, 128], f16)
        nc.sync.dma_start(ones, a["ones"])
        wk = singles.tile([128, 2, 512], f16)
        nc.sync.dma_start(wk, a["wkt"].rearrange("c p j -> p c j"))
        wq = singles.tile([128, 2, 512], f16)
        nc.sync.dma_start(wq, a["wqt"].rearrange("c p j -> p c j"))
        wv = singles.tile([128, 2, 1024], f16)
        bvs = singles.tile([128, H], f32)
        wp = singles.tile([128, 8, OUT], f16)
        bps = singles.tile([128, OUT], f32)
        ebt = singles.tile([128, H, NCH, NQ], f16)

        xt_p = ctx.enter_context(tc.tile_pool(name="xt", bufs=2))
        xst_p = ctx.enter_context(tc.tile_pool(name="xst", bufs=2))
        kt_p = ctx.enter_context(tc.tile_pool(name="kt", bufs=2))
        v_p = ctx.enter_context(tc.tile_pool(name="v", bufs=2))
        qt_p = ctx.enter_context(tc.tile_pool(name="qt", bufs=2))
        e_p = ctx.enter_context(tc.tile_pool(name="e", bufs=20))
        acc_p = ctx.enter_context(tc.tile_pool(name="acc", bufs=3))
        sacc_p = ctx.enter_context(tc.tile_pool(name="sacc", bufs=3))
        rc_p = ctx.enter_context(tc.tile_pool(name="rc", bufs=3))
        ot_p = ctx.enter_context(tc.tile_pool(name="ot", bufs=1))
        hs_p = ctx.enter_context(tc.tile_pool(name="hs", bufs=2))
        ob_p = ctx.enter_context(tc.tile_pool(name="ob", bufs=2))
        # scores pool: consumed ONLY by exp (ACT) so the sc->exp rotation never
        # chains behind copies; unit pool: proj/outproj psums, copy consumers
        ps_work = ctx.enter_context(tc.tile_pool(name="ps_work", bufs=2, space="PSUM"))
        ps_unit = ctx.enter_context(tc.tile_pool(name="ps_unit", bufs=2, space="PSUM"))
        ps_o = ctx.enter_context(tc.tile_pool(name="ps_o", bufs=1, space="PSUM"))
        ps_sum = ctx.enter_context(tc.tile_pool(name="ps_sum", bufs=1, space="PSUM"))

        _wt_n = [0]

        def work_tile():
            _wt_n[0] += 1
            return ps_work.tile([128, GRP, 512], f32, tag="w", name=f"wt{_wt_n[0]}")

        def unit_tile(alt=False):
            _wt_n[0] += 1
            if alt:
                # batch-0 warmup: nothing competes for the scores pool yet,
                # so borrow it to double the proj-psum rotation depth
                return ps_work.tile([128, GRP, 512], f32, tag="w",
                                    name=f"ut{_wt_n[0]}")[:, 0, :]
            return ps_unit.tile([128, 512], f32, tag="u", name=f"ut{_wt_n[0]}")

        _cp_n = [0]

        def copy_engine():
            # ~60/40 ACT/DVE keeps the two psum-capable engines level
            _cp_n[0] += 1
            return nc.scalar.copy if _cp_n[0] % 3 < 1 else nc.vector.tensor_copy

        def emit_eb_dma():
            for h in range(H):
                nc.sync.dma_start(
                    ebt[:, h, :, :],
                    a["eb"][h].rearrange("c p q -> p c q"),
                )

        # --- projection emission for one batch, as a list of closures ---
        def proj_units(b, xt, xst, kt, vt, qt, first_batch=False):
            units = []

            def kt_unit(pr, ns, alt=False):
                def run():
                    n0 = ns * 512
                    nsz = min(512, N - n0)
                    ps = unit_tile(alt)
                    for cc in range(2):
                        nc.tensor.matmul(
                            ps[:, :nsz],
                            lhsT=wk[:, cc, pr * 128:(pr + 1) * 128],
                            rhs=xt[:, cc, n0:n0 + nsz],
                            start=(cc == 0), stop=(cc == 1),
                        )
                    copy_engine()(kt[:, pr, n0:n0 + nsz], ps[:, :nsz])
                return run

            def v_unit(cn, hf, alt=False):
                def run():
                    ps = unit_tile(alt)
                    for cc in range(2):
                        nc.tensor.matmul(
                            ps,
                            lhsT=xt[:, cc, cn * 128:(cn + 1) * 128],
                            rhs=wv[:, cc, hf * 512:(hf + 1) * 512],
                            start=(cc == 0), stop=(cc == 1),
                        )
                    copy_engine()(vt[:, cn, hf * 512:(hf + 1) * 512], ps)
                return run

            def q_unit(pr, alt=False):
                def run():
                    ps = unit_tile(alt)
                    for cc in range(2):
                        nc.tensor.matmul(
                            ps[:, :NQ],
                            lhsT=wq[:, cc, pr * 128:(pr + 1) * 128],
                            rhs=xst[:, cc, :],
                            start=(cc == 0), stop=(cc == 1),
                        )
                    nc.scalar.activation(
                        qt[:, pr, :], ps[:, :NQ], AF.Identity,
                        bias=bqs[:, pr:pr + 1],
                    )
                return run

            i = [0]

            def alt():
                i[0] += 1
                return first_batch and i[0] % 2 == 0

            for pr in range(4):
                for ns in range(3):
                    units.append(kt_unit(pr, ns, alt()))
            for cn in range(NCH):
                for hf in range(2):
                    units.append(v_unit(cn, hf, alt()))
            for pr in range(4):
                units.append(q_unit(pr, alt()))
            return units

        # --- per-batch state for the software pipeline ---
        def head_attention(b, kt, vt, qt, ot, units_next):
            """Head loop with one-head o-matmul lag + proj interleaving."""
            nunits = len(units_next)
            consumed = [0]

            def consume(k):
                while consumed[0] < min(nunits, k):
                    units_next[consumed[0]]()
                    consumed[0] += 1

            e_tiles = [[None] * NGR for _ in range(H)]
            po_tiles = [None] * H
            rc_tiles = [None] * H
            psm_tiles = [None] * H

            def emit_o(h, g):
                po = po_tiles[h]
                for j in range(GRP):
                    c = GRP * g + j
                    nc.tensor.matmul(
                        po,
                        lhsT=vt[:, c, h * 128:(h + 1) * 128],
                        rhs=e_tiles[h][g][:, j, :],
                        start=(c == 0), stop=(c == NCH - 1),
                    )

            def emit_norm(h):
                # o = po/sums + bv ; hswish pairs after odd heads
                oh = ot[:, h, :]
                nc.vector.tensor_tensor(oh, po_tiles[h], rc_tiles[h], ALU.mult)
                nc.vector.tensor_scalar_add(oh, oh, bvs[:, h:h + 1])

            def emit_hswish(h0, t):
                # t = o*relu6(o+3), 1/6 folded into wp; two heads at once
                o2 = ot[:, h0:h0 + 2, :]
                t2 = t[:, h0:h0 + 2, :]
                nc.vector.tensor_scalar(t2, o2, 3.0, 6.0, ALU.add, ALU.min)
                nc.vector.tensor_scalar(t2, t2, 0.0, None, ALU.max)
                nc.vector.tensor_tensor(t2, t2, o2, ALU.mult)

            # hswish output doubles as out-proj lhsT for a batch PAIR:
            # q merged to 640 = 5x128 exact (vs 2.5x128 per batch)
            if b % 2 == 0:
                t_pair = hs_p.tile([128, H, 2, NQ], f16, tag="t", name=f"t_{b}")
                ts[b] = t_pair
            else:
                t_pair = ts[b - 1]
            t = t_pair[:, :, b % 2, :]
            acc_tiles = [None] * H

            def head_tail(h):
                # everything past the in-loop sum-tree for head h, emitted one
                # head late so no engine waits on the exp->mult->add chain
                acc = acc_tiles[h]
                nc.vector.tensor_tensor(acc, acc, e_tiles[h][NGR - 2], ALU.add)
                nc.vector.tensor_tensor(acc, acc, e_tiles[h][NGR - 1], ALU.add)
                sacc = sacc_p.tile([128, NQ], f16, tag="sacc",
                                   name=f"sacc{b}_{h}")
                nc.vector.tensor_tensor(sacc, acc[:, 0, :], acc[:, 1, :], ALU.add)
                psm = ps_sum.tile([128, NQ], f32, tag="psm", name=f"psm{b}_{h}")
                nc.tensor.matmul(psm, lhsT=ones, rhs=sacc, start=True, stop=True)
                rc_tiles[h] = rc_p.tile([128, NQ], f16, tag="rc", name=f"rc{b}_{h}")
                nc.vector.reciprocal(rc_tiles[h], psm)
                emit_norm(h)
                if h % 2 == 1:
                    emit_hswish(h - 1, t)

            for h in range(H):
                pr, p0 = h // 2, 64 * (h % 2)
                po_tiles[h] = ps_o.tile([128, NQ], f32, tag="po", name=f"po{b}_{h}")
                acc = acc_p.tile([128, GRP, NQ], f16, tag="acc", name=f"acc{b}_{h}")
                acc_tiles[h] = acc
                for g in range(NGR):
                    # READY work first: in-order engines, so anything emitted
                    # after the (dependency-blocked) scores would stall behind
                    # them. o-matmuls of head h-1 and proj units have ancient
                    # deps and fill the PE while exp/mult of g-2..g-1 complete.
                    if h > 0:
                        emit_o(h - 1, g)
                        consume((nunits * (h * NGR + g + 1)) // (H * NGR))
                    else:
                        consume(3 * (g + 1))
                    sg = work_tile()
                    for j in range(GRP):
                        c = GRP * g + j
                        nc.tensor.matmul(
                            sg[:, j, :NQ],
                            lhsT=kt[p0:p0 + 64, pr, c * 128:(c + 1) * 128],
                            rhs=qt[p0:p0 + 64, pr, :],
                            start=True, stop=True,
                        )
                    e = e_p.tile([128, GRP, NQ], f16, tag="e", name=f"e{b}_{h}_{g}")
                    e_tiles[h][g] = e
                    nc.scalar.activation(e, sg[:, :, :NQ], AF.Exp, bias=shiftc[:, 0:1])
                    # bias-mult is SBUF-only: Pool takes the middle groups,
                    # DVE the first/last
                    eng = nc.vector if g in (0, NGR - 1) else nc.gpsimd
                    eng.tensor_tensor(
                        e, e, ebt[:, h, GRP * g:GRP * (g + 1), :], ALU.mult)
                    # sums tree on DVE, two groups STALE so the add never
                    # waits on a Pool mult in flight (e0 is summed by PE
                    # ones-matmuls in head_tail instead)
                    if g == 3:
                        nc.vector.tensor_tensor(
                            acc, e_tiles[h][0], e_tiles[h][1], ALU.add)
                    elif g == 4:
                        nc.vector.tensor_tensor(
                            acc, acc, e_tiles[h][2], ALU.add)
                if h > 0:
                    head_tail(h - 1)
            # drain last head
            for g in range(NGR):
                emit_o(H - 1, g)
            head_tail(H - 1)
            consume(nunits)

        # --- main loop over batches ---
        kts, vts, qts, ts = {}, {}, {}, {}

        def alloc_and_dma(b):
            xt = xt_p.tile([128, 2, N], f16)
            for ns in range(3):
                n0 = ns * 512
                nsz = min(512, N - n0)
                nc.sync.dma_start(
                    xt[:, :, n0:n0 + nsz],
                    a["xt"][b, :, :, n0:n0 + nsz].rearrange("c p n -> p c n"),
                )
            xst = xst_p.tile([128, 2, NQ], f16)
            nc.sync.dma_start(xst, a["xst"][b].rearrange("c p n -> p c n"))
            kts[b] = kt_p.tile([128, 4, N], f16, tag="kt", name=f"kt{b}")
            vts[b] = v_p.tile([128, NCH, 1024], f16, tag="vt", name=f"vt{b}")
            qts[b] = qt_p.tile([128, 4, NQ], f16, tag="qt", name=f"qt{b}")
            return xt, xst

        def outproj_units(b):
            # output projection for batch pair (b-1, b), q merged to 640;
            # returned as closures consumed inside the NEXT batch's head loop
            out_flat = out_ap.rearrange("b q o -> (b q) o")
            t_pair = ts[b - 1]

            def qc_unit(qc):
                def run():
                    r0 = (b - 1) * NQ + qc * 128
                    ps = unit_tile()
                    for dc in range(8):
                        nc.tensor.matmul(
                            ps[:, :OUT],
                            lhsT=t_pair[:, dc, :, :].rearrange(
                                "p bb q -> p (bb q)")[:, qc * 128:(qc + 1) * 128],
                            rhs=wp[:, dc, :],
                            start=(dc == 0), stop=(dc == 7),
                        )
                    ob = ob_p.tile([128, OUT], f32)
                    nc.vector.tensor_tensor(ob, ps[:, :OUT], bps, ALU.add)
                    nc.sync.dma_start(out_flat[r0:r0 + 128, :], ob)
                return run

            return [qc_unit(qc) for qc in range(5)]

        xts = {}
        xts[0] = alloc_and_dma(0)
        # weights needed later in batch 0: emit DMA behind the first x tiles
        nc.sync.dma_start(wv, a["wvt"].rearrange("c p j -> p c j"))
        nc.sync.dma_start(bvs, a["bv"])
        emit_eb_dma()
        nc.sync.dma_start(wp, a["wpt"].rearrange("c p j -> p c j"))
        nc.sync.dma_start(bps, a["bp"])

        units0 = proj_units(0, *xts[0], kts[0], vts[0], qts[0], first_batch=True)
        for u in units0:
            u()
        carry_out = []
        for b in range(BPC):
            if b + 1 < BPC:
                xts[b + 1] = alloc_and_dma(b + 1)
                units_next = proj_units(
                    b + 1, *xts[b + 1], kts[b + 1], vts[b + 1], qts[b + 1])
            else:
                units_next = []
            units = list(units_next)
            # weave last pair's out-proj into the early-middle of this batch
            for i, u in enumerate(carry_out):
                pos = min(len(units), 6 + 7 * i)
                units.insert(pos, u)
            head_attention(b, kts[b], vts[b], qts[b],
                           ot_p.tile([128, H, NQ], f16, tag="ot", name=f"ot{b}"),
                           units)
            carry_out = outproj_units(b) if b % 2 == 1 else []
        for u in carry_out:
            u()


def build():
    import concourse.mybir as mybir
    import concourse.tile as tile
    from concourse import bacc

    nc = bacc.Bacc("TRN2", target_bir_lowering=False, debug=False)
    f32, f16 = mybir.dt.float32, mybir.dt.float16
    a = {}

    def din(name, shape, dt=f32):
        a[name] = nc.dram_tensor(name, shape, dt, kind="ExternalInput").ap()

    din("xt", [BPC, 2, 128, N], f16)
    din("xst", [BPC, 2, 128, NQ], f16)
    din("wkt", [2, 128, 512], f16)
    din("wvt", [2, 128, 1024], f16)
    din("wqt", [2, 128, 512], f16)
    din("wpt", [8, 128, OUT], f16)
    din("bq", [128, 4])
    din("bv", [128, H])
    din("bp", [128, OUT])
    din("eb", [H, NCH, 128, NQ], f16)
    din("ones", [128, 128], f16)
    out_ap = nc.dram_tensor("out", [BPC, NQ, OUT], f32, kind="ExternalOutput").ap()

    with tile.TileContext(nc) as tc:
        _body(tc, a, out_ap)
    nc.compile()
    return nc


_NC_CACHE = None


def _get_nc():
    global _NC_CACHE
    if _NC_CACHE is None:
        _NC_CACHE = build()
    return _NC_CACHE


def kernel(**inputs):
    from concourse.bass_utils import run_bass_kernel_spmd

    in_maps = _prep(inputs)
    nc = _get_nc()
    res = run_bass_kernel_spmd(nc, in_maps, list(range(NCORES)))
    out = np.concatenate([res.results[i]["out"] for i in range(NCORES)], axis=0)
    return np.ascontiguousarray(out, dtype=np.float32)


if __name__ == "__main__":
    print("smoke: building bass module...")
    nc = build()
    print("built ok:", sum(len(bb.instructions) for bb in nc.m.functions[0].blocks),
          "instructions")


# revision 72
# speedup vs baseline: 1.4274x; 1.0597x over previous
"""AttentionSubsample Trainium2 kernel (v2).

Full (unsharded) inputs in, full output out. Data-parallel over batch:
32 batches -> 8 NeuronCores x 4 batches each. Weights/biases replicated.

Key structure (vs v1 baseline at 275us; cost model now ~227us):
  - All non-PSUM tensors fp16 (same PE rate as bf16/f32r, 2x DVE modes,
    half the SBUF/DMA). Also ~7x more accurate than the bf16 v1.
  - Relative-position bias applied MULTIPLICATIVELY after exp:
    exp(s+b) = exp(s)*exp(b). exp(b) is precomputed on host, resident in
    SBUF (6.5MB, loaded once), applied with one tensor_tensor per group
    (mostly on the otherwise-idle Pool engine, DVE for first/last group)
    -- removes ~114us of f32 PSUM bias adds from DVE and the 26MB bias
    DMA stream entirely.
  - Softmax sums off the PE: e-chunk tree-accumulate on DVE (fp16, 2x,
    deps kept >=2 groups stale) + ONE ones-matmul per head on the
    accumulated [128,NQ] tile (PE 2560 cyc/batch instead of 25600).
  - k-channel BN bias dropped: softmax over n is invariant to per-q
    shifts, and (k+bk).q = k.q + bk.q[q] is exactly such a shift.
  - exp computed with a global -3 logit shift (ACT bias operand);
    softmax is exactly shift-invariant, and the shift keeps exp/sums
    inside fp16 range (logit max ~9 on this distribution).
  - PSUM: scores rotate through a dedicated 2-buf pool whose ONLY
    consumer is exp (ACT), so the sc->exp cadence never chains behind
    copies; projection/out-proj psums use a separate 2x1-bank pool
    (copy consumers); po/psm one bank each. Engines are in-order, so
    all ready work (o-matmuls of head h-1, proj units of batch b+1,
    out-proj of the previous pair) is emitted BEFORE each group's
    dependency-blocked scores, and the per-head tail (tree finish,
    ones-matmul, recip, normalize, hswish) is emitted one head late.
  - Batch-0 projections borrow the then-idle scores pool to double the
    warmup psum rotation depth.
"""

import sys

if "/opt/trn_rl_repo" not in sys.path:
    sys.path.insert(0, "/opt/trn_rl_repo")

import numpy as np

# --- problem constants (hardcoded, must match the grading reference) ---
B, N, C = 32, 1280, 256
H, KD, D = 8, 64, 128          # heads, key dim, value dim per head
NQ = 320                       # subsampled sequence length
OUT = 384
NCORES = 8
BPC = B // NCORES              # batches per core
EPS = 1e-5
NCH = N // 128                 # 10 n-chunks of 128
GRP = 2                        # chunks per score/exp group
NGR = NCH // GRP               # 5 groups per head
SHIFT = 3.0                    # global logit shift (exact for softmax)

_SUB_IDX = np.concatenate([
    (np.arange(32)[::2][:, None] * 32 + np.arange(32)[::2][None, :]).reshape(-1),
    1024 + (np.arange(16)[::2][:, None] * 16 + np.arange(16)[::2][None, :]).reshape(-1),
])  # [320] subsample row gather


def _prep(inputs):
    """Host-side: fold BN into weights, reorder channels, shard over cores."""
    f32 = np.float32
    f16 = np.float16
    x = np.asarray(inputs["x"], f32)
    g_kv, b_kv = np.asarray(inputs["g_kv"], f32), np.asarray(inputs["b_kv"], f32)
    rm_kv, rv_kv = np.asarray(inputs["rm_kv"], f32), np.asarray(inputs["rv_kv"], f32)
    g_q, b_q = np.asarray(inputs["g_q"], f32), np.asarray(inputs["b_q"], f32)
    rm_q, rv_q = np.asarray(inputs["rm_q"], f32), np.asarray(inputs["rv_q"], f32)
    g_p, b_p = np.asarray(inputs["g_p"], f32), np.asarray(inputs["b_p"], f32)
    rm_p, rv_p = np.asarray(inputs["rm_p"], f32), np.asarray(inputs["rv_p"], f32)
    W_kv = np.asarray(inputs["W_kv"], f32)
    W_q = np.asarray(inputs["W_q"], f32)
    W_p = np.asarray(inputs["W_p"], f32)
    attn_bias = np.asarray(inputs["attn_bias"], f32)
    bias_idxs = np.asarray(inputs["bias_idxs"])

    s_kv = g_kv / np.sqrt(rv_kv + EPS)
    Wkv_f = W_kv * s_kv[:, None]
    bkv_f = b_kv - rm_kv * s_kv
    kidx = np.concatenate([np.arange(h * 192, h * 192 + KD) for h in range(H)])
    vidx = np.concatenate([np.arange(h * 192 + KD, (h + 1) * 192) for h in range(H)])
    # k-channel bias dropped: softmax-invariant (see module docstring)
    wkt = np.ascontiguousarray(Wkv_f[kidx].T).reshape(2, 128, 512).astype(f16)
    wvt = np.ascontiguousarray(Wkv_f[vidx].T).reshape(2, 128, 1024).astype(f16)
    bvd = np.ascontiguousarray(bkv_f[vidx].reshape(8, 128).T)          # [128, H]

    scale = KD ** -0.5
    s_q = g_q / np.sqrt(rv_q + EPS)
    wqt = np.ascontiguousarray((W_q * (s_q * scale)[:, None]).T).reshape(
        2, 128, 512).astype(f16)
    bq = np.ascontiguousarray(((b_q - rm_q * s_q) * scale).reshape(4, 128).T)

    s_p = g_p / np.sqrt(rv_p + EPS)
    wpt = np.ascontiguousarray((W_p * s_p[:, None]).T / 6.0).reshape(
        8, 128, OUT).astype(f16)
    bp = np.ascontiguousarray(np.broadcast_to(b_p - rm_p * s_p, (128, OUT)))

    biasT = attn_bias[:, bias_idxs].transpose(0, 2, 1)                 # [H, N, NQ]
    eb = np.exp(biasT).reshape(H, NCH, 128, NQ).astype(f16)

    xs = x[:, _SUB_IDX, :]                                             # [B, NQ, C]
    in_maps = []
    for i in range(NCORES):
        sl = slice(i * BPC, (i + 1) * BPC)
        xt = np.ascontiguousarray(
            x[sl].transpose(0, 2, 1).reshape(BPC, 2, 128, N).astype(f16))
        xst = np.ascontiguousarray(
            xs[sl].transpose(0, 2, 1).reshape(BPC, 2, 128, NQ).astype(f16))
        in_maps.append({
            "xt": xt, "xst": xst,
            "wkt": wkt, "wvt": wvt, "wqt": wqt, "wpt": wpt,
            "bq": bq, "bv": bvd, "bp": bp,
            "eb": eb, "ones": np.ones((128, 128), f16),
        })
    return in_maps


def _body(tc, a, out_ap):
    import concourse.bass as bass  # noqa: F401
    import concourse.mybir as mybir
    from contextlib import ExitStack

    nc = tc.nc
    f32 = mybir.dt.float32
    f16 = mybir.dt.float16
    AF = mybir.ActivationFunctionType
    ALU = mybir.AluOpType

    with ExitStack() as ctx:
        ctx.enter_context(
            nc.allow_low_precision(reason="fp16 attention path is deliberate; "
                                   "verified against fp32 reference")
        )
        singles = ctx.enter_context(tc.tile_pool(name="singles", bufs=1))
        shiftc = singles.tile([128, 1], f32)
        nc.gpsimd.memset(shiftc, -SHIFT)
        bqs = singles.tile([128, 4], f32)
        nc.sync.dma_start(bqs, a["bq"])
        ones = singles.tile([128, 128], f16)
        nc.sync.dma_start(ones, a["ones"])
        wk = singles.tile([128, 2, 512], f16)
        nc.sync.dma_start(wk, a["wkt"].rearrange("c p j -> p c j"))
        wq = singles.tile([128, 2, 512], f16)
        nc.sync.dma_start(wq, a["wqt"].rearrange("c p j -> p c j"))
        wv = singles.tile([128, 2, 1024], f16)
        bvs = singles.tile([128, H], f32)
        wp = singles.tile([128, 8, OUT], f16)
        bps = singles.tile([128, OUT], f32)
        ebt = singles.tile([128, H, NCH, NQ], f16)

        xt_p = ctx.enter_context(tc.tile_pool(name="xt", bufs=2))
        xst_p = ctx.enter_context(tc.tile_pool(name="xst", bufs=2))
        kt_p = ctx.enter_context(tc.tile_pool(name="kt", bufs=2))
        v_p = ctx.enter_context(tc.tile_pool(name="v", bufs=2))
        qt_p = ctx.enter_context(tc.tile_pool(name="qt", bufs=2))
        e_p = ctx.enter_context(tc.tile_pool(name="e", bufs=20))
        acc_p = ctx.enter_context(tc.tile_pool(name="acc", bufs=3))
        rc_p = ctx.enter_context(tc.tile_pool(name="rc", bufs=3))
        ot_p = ctx.enter_context(tc.tile_pool(name="ot", bufs=1))
        hs_p = ctx.enter_context(tc.tile_pool(name="hs", bufs=2))
        ob_p = ctx.enter_context(tc.tile_pool(name="ob", bufs=2))
        orw_p = ctx.enter_context(tc.tile_pool(name="orw", bufs=2))
        # scores pool: consumed ONLY by exp (ACT) so the sc->exp rotation never
        # chains behind copies; unit pool: proj/outproj psums, copy consumers
        ps_work = ctx.enter_context(tc.tile_pool(name="ps_work", bufs=2, space="PSUM"))
        ps_unit = ctx.enter_context(tc.tile_pool(name="ps_unit", bufs=2, space="PSUM"))
        ps_o = ctx.enter_context(tc.tile_pool(name="ps_o", bufs=1, space="PSUM"))
        ps_sum = ctx.enter_context(tc.tile_pool(name="ps_sum", bufs=1, space="PSUM"))

        _wt_n = [0]

        def work_tile():
            _wt_n[0] += 1
            return ps_work.tile([128, GRP, 512], f32, tag="w", name=f"wt{_wt_n[0]}")

        def unit_tile(alt=False):
            _wt_n[0] += 1
            if alt:
                # batch-0 warmup: nothing competes for the scores pool yet,
                # so borrow it to double the proj-psum rotation depth
                return ps_work.tile([128, GRP, 512], f32, tag="w",
                                    name=f"ut{_wt_n[0]}")[:, 0, :]
            return ps_unit.tile([128, 512], f32, tag="u", name=f"ut{_wt_n[0]}")

        _cp_n = [0]

        def copy_engine():
            # ~60/40 ACT/DVE keeps the two psum-capable engines level
            _cp_n[0] += 1
            return nc.scalar.copy if _cp_n[0] % 3 < 1 else nc.vector.tensor_copy

        def emit_eb_dma():
            for h in range(H):
                nc.sync.dma_start(
                    ebt[:, h, :, :],
                    a["eb"][h].rearrange("c p q -> p c q"),
                )

        # --- projection emission for one batch, as a list of closures ---
        def proj_units(b, xt, xst, kt, vt, qt, first_batch=False):
            units = []

            def kt_unit(pr, ns, alt=False):
                def run():
                    n0 = ns * 512
                    nsz = min(512, N - n0)
                    ps = unit_tile(alt)
                    for cc in range(2):
                        nc.tensor.matmul(
                            ps[:, :nsz],
                            lhsT=wk[:, cc, pr * 128:(pr + 1) * 128],
                            rhs=xt[:, cc, n0:n0 + nsz],
                            start=(cc == 0), stop=(cc == 1),
                        )
                    copy_engine()(kt[:, pr, n0:n0 + nsz], ps[:, :nsz])
                return run

            def v_unit(cn, hf, alt=False):
                def run():
                    ps = unit_tile(alt)
                    for cc in range(2):
                        nc.tensor.matmul(
                            ps,
                            lhsT=xt[:, cc, cn * 128:(cn + 1) * 128],
                            rhs=wv[:, cc, hf * 512:(hf + 1) * 512],
                            start=(cc == 0), stop=(cc == 1),
                        )
                    copy_engine()(vt[:, cn, hf * 512:(hf + 1) * 512], ps)
                return run

            def q_unit(pr, alt=False):
                def run():
                    ps = unit_tile(alt)
                    for cc in range(2):
                        nc.tensor.matmul(
                            ps[:, :NQ],
                            lhsT=wq[:, cc, pr * 128:(pr + 1) * 128],
                            rhs=xst[:, cc, :],
                            start=(cc == 0), stop=(cc == 1),
                        )
                    nc.scalar.activation(
                        qt[:, pr, :], ps[:, :NQ], AF.Identity,
                        bias=bqs[:, pr:pr + 1],
                    )
                return run

            i = [0]

            def alt():
                i[0] += 1
                return first_batch and i[0] % 2 == 0

            for pr in range(4):
                for ns in range(3):
                    units.append(kt_unit(pr, ns, alt()))
            for cn in range(NCH):
                for hf in range(2):
                    units.append(v_unit(cn, hf, alt()))
            for pr in range(4):
                units.append(q_unit(pr, alt()))
            return units

        # --- per-batch state for the software pipeline ---
        def head_attention(b, kt, vt, qt, ot, units_next):
            """Head loop with one-head o-matmul lag + proj interleaving."""
            nunits = len(units_next)
            consumed = [0]

            def consume(k):
                while consumed[0] < min(nunits, k):
                    units_next[consumed[0]]()
                    consumed[0] += 1

            e_tiles = [[None] * NGR for _ in range(H)]
            po_tiles = [None] * H
            orw_tiles = [None] * H
            rc_tiles = [None] * H
            psm_tiles = [None] * H

            def emit_o(h, g):
                po = po_tiles[h]
                for j in range(GRP):
                    c = GRP * g + j
                    nc.tensor.matmul(
                        po,
                        lhsT=vt[:, c, h * 128:(h + 1) * 128],
                        rhs=e_tiles[h][g][:, j, :],
                        start=(c == 0), stop=(c == NCH - 1),
                    )

            def emit_oraw(h):
                # po -> sbuf on ACT (frees the po bank fast; runs while DVE
                # finishes the sum tree)
                oraw = orw_p.tile([128, NQ], f16, tag="orw", name=f"orw{b}_{h}")
                nc.scalar.copy(oraw, po_tiles[h])
                orw_tiles[h] = oraw

            def emit_norm(h):
                # normalize multiply in DVE 2x mode off-PSUM
                oh = ot[:, h, :]
                nc.vector.tensor_tensor(oh, orw_tiles[h], rc_tiles[h], ALU.mult)
                nc.vector.tensor_scalar_add(oh, oh, bvs[:, h:h + 1])

            def emit_hswish(h0, t):
                # t = o*relu6(o+3), 1/6 folded into wp; two heads at once
                o2 = ot[:, h0:h0 + 2, :]
                t2 = t[:, h0:h0 + 2, :]
                nc.vector.tensor_scalar(t2, o2, 3.0, 6.0, ALU.add, ALU.min)
                nc.vector.tensor_scalar(t2, t2, 0.0, None, ALU.max)
                nc.vector.tensor_tensor(t2, t2, o2, ALU.mult)

            # hswish output doubles as out-proj lhsT for a batch PAIR:
            # q merged to 640 = 5x128 exact (vs 2.5x128 per batch)
            if b % 2 == 0:
                t_pair = hs_p.tile([128, H, 2, NQ], f16, tag="t", name=f"t_{b}")
                ts[b] = t_pair
            else:
                t_pair = ts[b - 1]
            t = t_pair[:, :, b % 2, :]
            acc_tiles = [None] * H

            def head_tail(h):
                # everything past the in-loop sum-tree for head h, emitted one
                # head late so no engine waits on the exp->mult->add chain
                emit_oraw(h)
                acc = acc_tiles[h]
                nc.vector.tensor_tensor(acc, acc, e_tiles[h][NGR - 2], ALU.add)
                nc.vector.tensor_tensor(acc, acc, e_tiles[h][NGR - 1], ALU.add)
                psm = ps_sum.tile([128, NQ], f32, tag="psm", name=f"psm{b}_{h}")
                nc.tensor.matmul(psm, lhsT=ones, rhs=acc[:, 0, :],
                                 start=True, stop=False)
                nc.tensor.matmul(psm, lhsT=ones, rhs=acc[:, 1, :],
                                 start=False, stop=True)
                rc_tiles[h] = rc_p.tile([128, NQ], f16, tag="rc", name=f"rc{b}_{h}")
                nc.vector.reciprocal(rc_tiles[h], psm)
                emit_norm(h)
                if h % 2 == 1:
                    emit_hswish(h - 1, t)

            for h in range(H):
                pr, p0 = h // 2, 64 * (h % 2)
                po_tiles[h] = ps_o.tile([128, NQ], f32, tag="po", name=f"po{b}_{h}")
                acc = acc_p.tile([128, GRP, NQ], f16, tag="acc", name=f"acc{b}_{h}")
                acc_tiles[h] = acc
                for g in range(NGR):
                    # READY work first: in-order engines, so anything emitted
                    # after the (dependency-blocked) scores would stall behind
                    # them. o-matmuls of head h-1 and proj units have ancient
                    # deps and fill the PE while exp/mult of g-2..g-1 complete.
                    if h > 0:
                        emit_o(h - 1, g)
                        consume((nunits * (h * NGR + g + 1)) // (H * NGR))
                    else:
                        consume(3 * (g + 1))
                    sg = work_tile()
                    for j in range(GRP):
                        c = GRP * g + j
                        nc.tensor.matmul(
                            sg[:, j, :NQ],
                            lhsT=kt[p0:p0 + 64, pr, c * 128:(c + 1) * 128],
                            rhs=qt[p0:p0 + 64, pr, :],
                            start=True, stop=True,
                        )
                    e = e_p.tile([128, GRP, NQ], f16, tag="e", name=f"e{b}_{h}_{g}")
                    e_tiles[h][g] = e
                    nc.scalar.activation(e, sg[:, :, :NQ], AF.Exp, bias=shiftc[:, 0:1])
                    # bias-mult is SBUF-only: Pool takes the middle groups,
                    # DVE the first/last
                    eng = nc.vector if g in (0, NGR - 1) else nc.gpsimd
                    eng.tensor_tensor(
                        e, e, ebt[:, h, GRP * g:GRP * (g + 1), :], ALU.mult)
                    # sums tree on DVE, two groups STALE so the add never
                    # waits on a Pool mult in flight (e0 is summed by PE
                    # ones-matmuls in head_tail instead)
                    if g == 3:
                        nc.vector.tensor_tensor(
                            acc, e_tiles[h][0], e_tiles[h][1], ALU.add)
                    elif g == 4:
                        nc.vector.tensor_tensor(
                            acc, acc, e_tiles[h][2], ALU.add)
                if h > 0:
                    head_tail(h - 1)
            # drain last head
            for g in range(NGR):
                emit_o(H - 1, g)
            head_tail(H - 1)
            consume(nunits)

        # --- main loop over batches ---
        kts, vts, qts, ts = {}, {}, {}, {}

        def alloc_and_dma(b):
            xt = xt_p.tile([128, 2, N], f16)
            for ns in range(3):
                n0 = ns * 512
                nsz = min(512, N - n0)
                nc.sync.dma_start(
                    xt[:, :, n0:n0 + nsz],
                    a["xt"][b, :, :, n0:n0 + nsz].rearrange("c p n -> p c n"),
                )
            xst = xst_p.tile([128, 2, NQ], f16)
            nc.sync.dma_start(xst, a["xst"][b].rearrange("c p n -> p c n"))
            kts[b] = kt_p.tile([128, 4, N], f16, tag="kt", name=f"kt{b}")
            vts[b] = v_p.tile([128, NCH, 1024], f16, tag="vt", name=f"vt{b}")
            qts[b] = qt_p.tile([128, 4, NQ], f16, tag="qt", name=f"qt{b}")
            return xt, xst

        def outproj_units(b):
            # output projection for batch pair (b-1, b), q merged to 640;
            # returned as closures consumed inside the NEXT batch's head loop
            out_flat = out_ap.rearrange("b q o -> (b q) o")
            t_pair = ts[b - 1]

            def qc_unit(qc):
                def run():
                    r0 = (b - 1) * NQ + qc * 128
                    ps = unit_tile()
                    for dc in range(8):
                        nc.tensor.matmul(
                            ps[:, :OUT],
                            lhsT=t_pair[:, dc, :, :].rearrange(
                                "p bb q -> p (bb q)")[:, qc * 128:(qc + 1) * 128],
                            rhs=wp[:, dc, :],
                            start=(dc == 0), stop=(dc == 7),
                        )
                    ob = ob_p.tile([128, OUT], f32)
                    nc.vector.tensor_tensor(ob, ps[:, :OUT], bps, ALU.add)
                    nc.sync.dma_start(out_flat[r0:r0 + 128, :], ob)
                return run

            return [qc_unit(qc) for qc in range(5)]

        xts = {}
        xts[0] = alloc_and_dma(0)
        # weights needed later in batch 0: emit DMA behind the first x tiles
        nc.sync.dma_start(wv, a["wvt"].rearrange("c p j -> p c j"))
        nc.sync.dma_start(bvs, a["bv"])
        emit_eb_dma()
        nc.sync.dma_start(wp, a["wpt"].rearrange("c p j -> p c j"))
        nc.sync.dma_start(bps, a["bp"])

        units0 = proj_units(0, *xts[0], kts[0], vts[0], qts[0], first_batch=True)
        for u in units0:
            u()
        carry_out = []
        for b in range(BPC):
            if b + 1 < BPC:
                xts[b + 1] = alloc_and_dma(b + 1)
                units_next = proj_units(
                    b + 1, *xts[b + 1], kts[b + 1], vts[b + 1], qts[b + 1])
            else:
                units_next = []
            units = list(units_next)
            # weave last pair's out-proj into the early-middle of this batch
            for i, u in enumerate(carry_out):
                pos = min(len(units), 6 + 7 * i)
                units.insert(pos, u)
            head_attention(b, kts[b], vts[b], qts[b],
                           ot_p.tile([128, H, NQ], f16, tag="ot", name=f"ot{b}"),
                           units)
            carry_out = outproj_units(b) if b % 2 == 1 else []
        for u in carry_out:
            u()


def build():
    import concourse.mybir as mybir
    import concourse.tile as tile
    from concourse import bacc

    nc = bacc.Bacc("TRN2", target_bir_lowering=False, debug=False)
    f32, f16 = mybir.dt.float32, mybir.dt.float16
    a = {}

    def din(name, shape, dt=f32):
        a[name] = nc.dram_tensor(name, shape, dt, kind="ExternalInput").ap()

    din("xt", [BPC, 2, 128, N], f16)
    din("xst", [BPC, 2, 128, NQ], f16)
    din("wkt", [2, 128, 512], f16)
    din("wvt", [2, 128, 1024], f16)
    din("wqt", [2, 128, 512], f16)
    din("wpt", [8, 128, OUT], f16)
    din("bq", [128, 4])
    din("bv", [128, H])
    din("bp", [128, OUT])
    din("eb", [H, NCH, 128, NQ], f16)
    din("ones", [128, 128], f16)
    out_ap = nc.dram_tensor("out", [BPC, NQ, OUT], f32, kind="ExternalOutput").ap()

    with tile.TileContext(nc) as tc:
        _body(tc, a, out_ap)
    nc.compile()
    return nc


_NC_CACHE = None


def _get_nc():
    global _NC_CACHE
    if _NC_CACHE is None:
        _NC_CACHE = build()
    return _NC_CACHE


def kernel(**inputs):
    from concourse.bass_utils import run_bass_kernel_spmd

    in_maps = _prep(inputs)
    nc = _get_nc()
    res = run_bass_kernel_spmd(nc, in_maps, list(range(NCORES)))
    out = np.concatenate([res.results[i]["out"] for i in range(NCORES)], axis=0)
    return np.ascontiguousarray(out, dtype=np.float32)


if __name__ == "__main__":
    print("smoke: building bass module...")
    nc = build()
    print("built ok:", sum(len(bb.instructions) for bb in nc.m.functions[0].blocks),
          "instructions")


# revision 75
# speedup vs baseline: 1.4480x; 1.0144x over previous
"""AttentionSubsample Trainium2 kernel (v2).

Full (unsharded) inputs in, full output out. Data-parallel over batch:
32 batches -> 8 NeuronCores x 4 batches each. Weights/biases replicated.

Key structure (vs v1 baseline at 275us; cost model now ~214us):
  - All non-PSUM tensors fp16 (same PE rate as bf16/f32r, 2x DVE modes,
    half the SBUF/DMA). Also ~7x more accurate than the bf16 v1.
  - Relative-position bias applied MULTIPLICATIVELY after exp:
    exp(s+b) = exp(s)*exp(b). exp(b) is precomputed on host, resident in
    SBUF (6.5MB, loaded once), applied with one tensor_tensor per group
    (mostly on the otherwise-idle Pool engine, DVE for first/last group)
    -- removes ~114us of f32 PSUM bias adds from DVE and the 26MB bias
    DMA stream entirely.
  - Softmax sums off the PE: e-chunk tree-accumulate on DVE (fp16, 2x,
    deps kept >=2 groups stale) + ONE ones-matmul per head on the
    accumulated [128,NQ] tile (PE 2560 cyc/batch instead of 25600).
  - k-channel BN bias dropped: softmax over n is invariant to per-q
    shifts, and (k+bk).q = k.q + bk.q[q] is exactly such a shift.
  - exp computed with a global -3 logit shift (ACT bias operand);
    softmax is exactly shift-invariant, and the shift keeps exp/sums
    inside fp16 range (logit max ~9 on this distribution).
  - PSUM: scores rotate through a dedicated 2-buf pool whose ONLY
    consumer is exp (ACT), so the sc->exp cadence never chains behind
    copies; projection/out-proj psums use a separate 2x1-bank pool
    (copy consumers); po/psm one bank each. Engines are in-order, so
    all ready work (o-matmuls of head h-1, proj units of batch b+1,
    out-proj of the previous pair) is emitted BEFORE each group's
    dependency-blocked scores, and the per-head tail (tree finish,
    ones-matmul, recip, normalize, hswish) is emitted one head late.
  - Batch-0 projections borrow the then-idle scores pool to double the
    warmup psum rotation depth.
"""

import sys

if "/opt/trn_rl_repo" not in sys.path:
    sys.path.insert(0, "/opt/trn_rl_repo")

import numpy as np

# --- problem constants (hardcoded, must match the grading reference) ---
B, N, C = 32, 1280, 256
H, KD, D = 8, 64, 128          # heads, key dim, value dim per head
NQ = 320                       # subsampled sequence length
OUT = 384
NCORES = 8
BPC = B // NCORES              # batches per core
EPS = 1e-5
NCH = N // 128                 # 10 n-chunks of 128
GRP = 2                        # chunks per score/exp group
NGR = NCH // GRP               # 5 groups per head
SHIFT = 3.0                    # global logit shift (exact for softmax)

_SUB_IDX = np.concatenate([
    (np.arange(32)[::2][:, None] * 32 + np.arange(32)[::2][None, :]).reshape(-1),
    1024 + (np.arange(16)[::2][:, None] * 16 + np.arange(16)[::2][None, :]).reshape(-1),
])  # [320] subsample row gather


def _prep(inputs):
    """Host-side: fold BN into weights, reorder channels, shard over cores."""
    f32 = np.float32
    f16 = np.float16
    x = np.asarray(inputs["x"], f32)
    g_kv, b_kv = np.asarray(inputs["g_kv"], f32), np.asarray(inputs["b_kv"], f32)
    rm_kv, rv_kv = np.asarray(inputs["rm_kv"], f32), np.asarray(inputs["rv_kv"], f32)
    g_q, b_q = np.asarray(inputs["g_q"], f32), np.asarray(inputs["b_q"], f32)
    rm_q, rv_q = np.asarray(inputs["rm_q"], f32), np.asarray(inputs["rv_q"], f32)
    g_p, b_p = np.asarray(inputs["g_p"], f32), np.asarray(inputs["b_p"], f32)
    rm_p, rv_p = np.asarray(inputs["rm_p"], f32), np.asarray(inputs["rv_p"], f32)
    W_kv = np.asarray(inputs["W_kv"], f32)
    W_q = np.asarray(inputs["W_q"], f32)
    W_p = np.asarray(inputs["W_p"], f32)
    attn_bias = np.asarray(inputs["attn_bias"], f32)
    bias_idxs = np.asarray(inputs["bias_idxs"])

    s_kv = g_kv / np.sqrt(rv_kv + EPS)
    Wkv_f = W_kv * s_kv[:, None]
    bkv_f = b_kv - rm_kv * s_kv
    kidx = np.concatenate([np.arange(h * 192, h * 192 + KD) for h in range(H)])
    vidx = np.concatenate([np.arange(h * 192 + KD, (h + 1) * 192) for h in range(H)])
    # k-channel bias dropped: softmax-invariant (see module docstring)
    wkt = np.ascontiguousarray(Wkv_f[kidx].T).reshape(2, 128, 512).astype(f16)
    wvt = np.ascontiguousarray(Wkv_f[vidx].T).reshape(2, 128, 1024).astype(f16)
    bvd = np.ascontiguousarray(bkv_f[vidx].reshape(8, 128).T)          # [128, H]

    scale = KD ** -0.5
    s_q = g_q / np.sqrt(rv_q + EPS)
    wqt = np.ascontiguousarray((W_q * (s_q * scale)[:, None]).T).reshape(
        2, 128, 512).astype(f16)
    bq = np.ascontiguousarray(((b_q - rm_q * s_q) * scale).reshape(4, 128).T)

    s_p = g_p / np.sqrt(rv_p + EPS)
    wpt = np.ascontiguousarray((W_p * s_p[:, None]).T / 6.0).reshape(
        8, 128, OUT).astype(f16)
    bp = np.ascontiguousarray(np.broadcast_to(b_p - rm_p * s_p, (128, OUT)))

    biasT = attn_bias[:, bias_idxs].transpose(0, 2, 1)                 # [H, N, NQ]
    eb = np.exp(biasT).reshape(H, NCH, 128, NQ).astype(f16)

    xs = x[:, _SUB_IDX, :]                                             # [B, NQ, C]
    in_maps = []
    for i in range(NCORES):
        sl = slice(i * BPC, (i + 1) * BPC)
        xt = np.ascontiguousarray(
            x[sl].transpose(0, 2, 1).reshape(BPC, 2, 128, N).astype(f16))
        xst = np.ascontiguousarray(
            xs[sl].transpose(0, 2, 1).reshape(BPC, 2, 128, NQ).astype(f16))
        in_maps.append({
            "xt": xt, "xst": xst,
            "wkt": wkt, "wvt": wvt, "wqt": wqt, "wpt": wpt,
            "bq": bq, "bv": bvd, "bp": bp,
            "eb": eb, "ones": np.ones((128, 128), f16),
        })
    return in_maps


def _body(tc, a, out_ap):
    import concourse.bass as bass  # noqa: F401
    import concourse.mybir as mybir
    from contextlib import ExitStack

    nc = tc.nc
    f32 = mybir.dt.float32
    f16 = mybir.dt.float16
    AF = mybir.ActivationFunctionType
    ALU = mybir.AluOpType

    with ExitStack() as ctx:
        ctx.enter_context(
            nc.allow_low_precision(reason="fp16 attention path is deliberate; "
                                   "verified against fp32 reference")
        )
        singles = ctx.enter_context(tc.tile_pool(name="singles", bufs=1))
        shiftc = singles.tile([128, 1], f32)
        nc.gpsimd.memset(shiftc, -SHIFT)
        bqs = singles.tile([128, 4], f32)
        nc.sync.dma_start(bqs, a["bq"])
        ones = singles.tile([128, 128], f16)
        nc.sync.dma_start(ones, a["ones"])
        wk = singles.tile([128, 2, 512], f16)
        nc.sync.dma_start(wk, a["wkt"].rearrange("c p j -> p c j"))
        wq = singles.tile([128, 2, 512], f16)
        nc.sync.dma_start(wq, a["wqt"].rearrange("c p j -> p c j"))
        wv = singles.tile([128, 2, 1024], f16)
        bvs = singles.tile([128, H], f32)
        wp = singles.tile([128, 8, OUT], f16)
        bps = singles.tile([128, OUT], f32)
        ebt = singles.tile([128, H, NCH, NQ], f16)

        xt_p = ctx.enter_context(tc.tile_pool(name="xt", bufs=2))
        xst_p = ctx.enter_context(tc.tile_pool(name="xst", bufs=2))
        kt_p = ctx.enter_context(tc.tile_pool(name="kt", bufs=2))
        v_p = ctx.enter_context(tc.tile_pool(name="v", bufs=2))
        qt_p = ctx.enter_context(tc.tile_pool(name="qt", bufs=2))
        e_p = ctx.enter_context(tc.tile_pool(name="e", bufs=20))
        acc_p = ctx.enter_context(tc.tile_pool(name="acc", bufs=3))
        rc_p = ctx.enter_context(tc.tile_pool(name="rc", bufs=3))
        ot_p = ctx.enter_context(tc.tile_pool(name="ot", bufs=1))
        hs_p = ctx.enter_context(tc.tile_pool(name="hs", bufs=2))
        ob_p = ctx.enter_context(tc.tile_pool(name="ob", bufs=2))
        orw_p = ctx.enter_context(tc.tile_pool(name="orw", bufs=2))
        # scores pool: consumed ONLY by exp (ACT) so the sc->exp rotation never
        # chains behind copies; unit pool: proj/outproj psums, copy consumers
        ps_work = ctx.enter_context(tc.tile_pool(name="ps_work", bufs=2, space="PSUM"))
        ps_unit = ctx.enter_context(tc.tile_pool(name="ps_unit", bufs=2, space="PSUM"))
        ps_o = ctx.enter_context(tc.tile_pool(name="ps_o", bufs=1, space="PSUM"))
        ps_sum = ctx.enter_context(tc.tile_pool(name="ps_sum", bufs=1, space="PSUM"))

        _wt_n = [0]

        def work_tile():
            _wt_n[0] += 1
            return ps_work.tile([128, GRP, 512], f32, tag="w", name=f"wt{_wt_n[0]}")

        def unit_tile(alt=False):
            _wt_n[0] += 1
            if alt:
                # batch-0 warmup: nothing competes for the scores pool yet,
                # so borrow it to double the proj-psum rotation depth
                return ps_work.tile([128, GRP, 512], f32, tag="w",
                                    name=f"ut{_wt_n[0]}")[:, 0, :]
            return ps_unit.tile([128, 512], f32, tag="u", name=f"ut{_wt_n[0]}")

        _cp_n = [0]

        def copy_engine():
            # ~60/40 ACT/DVE keeps the two psum-capable engines level
            _cp_n[0] += 1
            return nc.scalar.copy if _cp_n[0] % 3 < 1 else nc.vector.tensor_copy

        def emit_eb_dma():
            for h in range(H):
                nc.sync.dma_start(
                    ebt[:, h, :, :],
                    a["eb"][h].rearrange("c p q -> p c q"),
                )

        # --- projection emission for one batch, as a list of closures ---
        def proj_units(b, xt, xst, kt, vt, qt, first_batch=False):
            units = []

            def kt_unit(pr, ns, alt=False):
                def run():
                    n0 = ns * 512
                    nsz = min(512, N - n0)
                    ps = unit_tile(alt)
                    for cc in range(2):
                        nc.tensor.matmul(
                            ps[:, :nsz],
                            lhsT=wk[:, cc, pr * 128:(pr + 1) * 128],
                            rhs=xt[:, cc, n0:n0 + nsz],
                            start=(cc == 0), stop=(cc == 1),
                        )
                    copy_engine()(kt[:, pr, n0:n0 + nsz], ps[:, :nsz])
                return run

            def v_unit(cn, hf, alt=False):
                def run():
                    ps = unit_tile(alt)
                    for cc in range(2):
                        nc.tensor.matmul(
                            ps,
                            lhsT=xt[:, cc, cn * 128:(cn + 1) * 128],
                            rhs=wv[:, cc, hf * 512:(hf + 1) * 512],
                            start=(cc == 0), stop=(cc == 1),
                        )
                    copy_engine()(vt[:, cn, hf * 512:(hf + 1) * 512], ps)
                return run

            def q_unit(pr, alt=False):
                def run():
                    ps = unit_tile(alt)
                    for cc in range(2):
                        nc.tensor.matmul(
                            ps[:, :NQ],
                            lhsT=wq[:, cc, pr * 128:(pr + 1) * 128],
                            rhs=xst[:, cc, :],
                            start=(cc == 0), stop=(cc == 1),
                        )
                    nc.scalar.activation(
                        qt[:, pr, :], ps[:, :NQ], AF.Identity,
                        bias=bqs[:, pr:pr + 1],
                    )
                return run

            i = [0]

            def alt():
                i[0] += 1
                return first_batch and i[0] % 2 == 0

            for pr in range(4):
                for ns in range(3):
                    units.append(kt_unit(pr, ns, alt()))
            for cn in range(NCH):
                for hf in range(2):
                    units.append(v_unit(cn, hf, alt()))
            for pr in range(4):
                units.append(q_unit(pr, alt()))
            return units

        # --- per-batch state for the software pipeline ---
        def head_attention(b, kt, vt, qt, ot, units_next):
            """Head loop with one-head o-matmul lag + proj interleaving."""
            nunits = len(units_next)
            consumed = [0]

            def consume(k):
                while consumed[0] < min(nunits, k):
                    units_next[consumed[0]]()
                    consumed[0] += 1

            e_tiles = [[None] * NGR for _ in range(H)]
            po_tiles = [None] * H
            orw_tiles = [None] * H
            rc_tiles = [None] * H
            psm_tiles = [None] * H

            def emit_o(h, g):
                po = po_tiles[h]
                for j in range(GRP):
                    c = GRP * g + j
                    nc.tensor.matmul(
                        po,
                        lhsT=vt[:, c, h * 128:(h + 1) * 128],
                        rhs=e_tiles[h][g][:, j, :],
                        start=(c == 0), stop=(c == NCH - 1),
                    )

            def emit_oraw(h):
                # po -> sbuf on ACT (frees the po bank fast; runs while DVE
                # finishes the sum tree)
                oraw = orw_p.tile([128, NQ], f16, tag="orw", name=f"orw{b}_{h}")
                nc.scalar.copy(oraw, po_tiles[h])
                orw_tiles[h] = oraw

            def emit_norm(h):
                # normalize multiply in DVE 2x mode off-PSUM
                oh = ot[:, h, :]
                nc.vector.tensor_tensor(oh, orw_tiles[h], rc_tiles[h], ALU.mult)
                nc.vector.tensor_scalar_add(oh, oh, bvs[:, h:h + 1])

            def emit_hswish(h0, t):
                # t = o*relu6(o+3), 1/6 folded into wp; two heads at once
                o2 = ot[:, h0:h0 + 2, :]
                t2 = t[:, h0:h0 + 2, :]
                nc.vector.tensor_scalar(t2, o2, 3.0, 6.0, ALU.add, ALU.min)
                nc.vector.tensor_scalar(t2, t2, 0.0, None, ALU.max)
                nc.vector.tensor_tensor(t2, t2, o2, ALU.mult)

            # hswish output doubles as out-proj lhsT for a batch PAIR:
            # q merged to 640 = 5x128 exact (vs 2.5x128 per batch)
            if b % 2 == 0:
                t_pair = hs_p.tile([128, H, 2, NQ], f16, tag="t", name=f"t_{b}")
                ts[b] = t_pair
            else:
                t_pair = ts[b - 1]
            t = t_pair[:, :, b % 2, :]
            acc_tiles = [None] * H

            def head_tail(h):
                # everything past the in-loop sum-tree for head h, emitted one
                # head late so no engine waits on the exp->mult->add chain
                emit_oraw(h)
                acc = acc_tiles[h]
                nc.vector.tensor_tensor(acc, acc, e_tiles[h][NGR - 2], ALU.add)
                nc.vector.tensor_tensor(acc, acc, e_tiles[h][NGR - 1], ALU.add)
                psm = ps_sum.tile([128, NQ], f32, tag="psm", name=f"psm{b}_{h}")
                nc.tensor.matmul(psm, lhsT=ones, rhs=acc[:, 0, :],
                                 start=True, stop=False)
                nc.tensor.matmul(psm, lhsT=ones, rhs=acc[:, 1, :],
                                 start=False, stop=True)
                rc_tiles[h] = rc_p.tile([128, NQ], f16, tag="rc", name=f"rc{b}_{h}")
                nc.vector.reciprocal(rc_tiles[h], psm)
                emit_norm(h)
                if h % 2 == 1:
                    emit_hswish(h - 1, t)

            for h in range(H):
                pr, p0 = h // 2, 64 * (h % 2)
                po_tiles[h] = ps_o.tile([128, NQ], f32, tag="po", name=f"po{b}_{h}")
                acc = acc_p.tile([128, GRP, NQ], f16, tag="acc", name=f"acc{b}_{h}")
                acc_tiles[h] = acc
                for g in range(NGR):
                    # READY work first: in-order engines, so anything emitted
                    # after the (dependency-blocked) scores would stall behind
                    # them. o-matmuls of head h-1 and proj units have ancient
                    # deps and fill the PE while exp/mult of g-2..g-1 complete.
                    if h > 0:
                        emit_o(h - 1, g)
                        consume((nunits * (h * NGR + g + 1)) // (H * NGR))
                    else:
                        consume(3 * (g + 1))
                    sg = work_tile()
                    for j in range(GRP):
                        c = GRP * g + j
                        nc.tensor.matmul(
                            sg[:, j, :NQ],
                            lhsT=kt[p0:p0 + 64, pr, c * 128:(c + 1) * 128],
                            rhs=qt[p0:p0 + 64, pr, :],
                            start=True, stop=True,
                        )
                    e = e_p.tile([128, GRP, NQ], f16, tag="e", name=f"e{b}_{h}_{g}")
                    e_tiles[h][g] = e
                    nc.scalar.activation(e, sg[:, :, :NQ], AF.Exp, bias=shiftc[:, 0:1])
                    # bias-mult is SBUF-only: Pool takes the middle groups,
                    # DVE the first/last
                    eng = nc.vector if g in (0, NGR - 1) else nc.gpsimd
                    eng.tensor_tensor(
                        e, e, ebt[:, h, GRP * g:GRP * (g + 1), :], ALU.mult)
                    # sums tree on DVE, two groups STALE so the add never
                    # waits on a Pool mult in flight (e0 is summed by PE
                    # ones-matmuls in head_tail instead)
                    if g == 3:
                        nc.vector.tensor_tensor(
                            acc, e_tiles[h][0], e_tiles[h][1], ALU.add)
                    elif g == 4:
                        nc.vector.tensor_tensor(
                            acc, acc, e_tiles[h][2], ALU.add)
                if h > 0:
                    head_tail(h - 1)
            # drain last head
            for g in range(NGR):
                emit_o(H - 1, g)
            head_tail(H - 1)
            consume(nunits)

        # --- main loop over batches ---
        kts, vts, qts, ts = {}, {}, {}, {}

        def alloc_and_dma(b):
            xt = xt_p.tile([128, 2, N], f16)
            for ns in range(3):
                n0 = ns * 512
                nsz = min(512, N - n0)
                nc.sync.dma_start(
                    xt[:, :, n0:n0 + nsz],
                    a["xt"][b, :, :, n0:n0 + nsz].rearrange("c p n -> p c n"),
                )
            xst = xst_p.tile([128, 2, NQ], f16)
            nc.sync.dma_start(xst, a["xst"][b].rearrange("c p n -> p c n"))
            kts[b] = kt_p.tile([128, 4, N], f16, tag="kt", name=f"kt{b}")
            vts[b] = v_p.tile([128, NCH, 1024], f16, tag="vt", name=f"vt{b}")
            qts[b] = qt_p.tile([128, 4, NQ], f16, tag="qt", name=f"qt{b}")
            return xt, xst

        def outproj_units(b):
            # output projection for batch pair (b-1, b), q merged to 640;
            # returned as closures consumed inside the NEXT batch's head loop
            out_flat = out_ap.rearrange("b q o -> (b q) o")
            t_pair = ts[b - 1]

            def qc_unit(qc):
                def run():
                    r0 = (b - 1) * NQ + qc * 128
                    ps = unit_tile()
                    for dc in range(8):
                        nc.tensor.matmul(
                            ps[:, :OUT],
                            lhsT=t_pair[:, dc, :, :].rearrange(
                                "p bb q -> p (bb q)")[:, qc * 128:(qc + 1) * 128],
                            rhs=wp[:, dc, :],
                            start=(dc == 0), stop=(dc == 7),
                        )
                    ob = ob_p.tile([128, OUT], f32)
                    nc.vector.tensor_tensor(ob, ps[:, :OUT], bps, ALU.add)
                    nc.sync.dma_start(out_flat[r0:r0 + 128, :], ob)
                return run

            return [qc_unit(qc) for qc in range(5)]

        xts = {}
        xts[0] = alloc_and_dma(0)
        # weights needed later in batch 0: emit DMA behind the first x tiles
        nc.sync.dma_start(wv, a["wvt"].rearrange("c p j -> p c j"))
        nc.sync.dma_start(bvs, a["bv"])
        emit_eb_dma()
        nc.sync.dma_start(wp, a["wpt"].rearrange("c p j -> p c j"))
        nc.sync.dma_start(bps, a["bp"])

        units0 = proj_units(0, *xts[0], kts[0], vts[0], qts[0], first_batch=True)
        for u in units0:
            u()
        carry_out = []
        for b in range(BPC):
            if b + 1 < BPC:
                xts[b + 1] = alloc_and_dma(b + 1)
                units_next = proj_units(
                    b + 1, *xts[b + 1], kts[b + 1], vts[b + 1], qts[b + 1])
            else:
                units_next = []
            units = list(units_next)
            # weave last pair's out-proj into the early-middle of this batch
            for i, u in enumerate(carry_out):
                pos = min(len(units), 6 + 7 * i)
                units.insert(pos, u)
            head_attention(b, kts[b], vts[b], qts[b],
                           ot_p.tile([128, H, NQ], f16, tag="ot", name=f"ot{b}"),
                           units)
            carry_out = outproj_units(b) if b % 2 == 1 else []
        for u in carry_out:
            u()


def build():
    import concourse.mybir as mybir
    import concourse.tile as tile
    from concourse import bacc

    nc = bacc.Bacc("TRN2", target_bir_lowering=False, debug=False)
    f32, f16 = mybir.dt.float32, mybir.dt.float16
    a = {}

    def din(name, shape, dt=f32):
        a[name] = nc.dram_tensor(name, shape, dt, kind="ExternalInput").ap()

    din("xt", [BPC, 2, 128, N], f16)
    din("xst", [BPC, 2, 128, NQ], f16)
    din("wkt", [2, 128, 512], f16)
    din("wvt", [2, 128, 1024], f16)
    din("wqt", [2, 128, 512], f16)
    din("wpt", [8, 128, OUT], f16)
    din("bq", [128, 4])
    din("bv", [128, H])
    din("bp", [128, OUT])
    din("eb", [H, NCH, 128, NQ], f16)
    din("ones", [128, 128], f16)
    out_ap = nc.dram_tensor("out", [BPC, NQ, OUT], f32, kind="ExternalOutput").ap()

    with tile.TileContext(nc) as tc:
        _body(tc, a, out_ap)
    nc.compile()
    return nc


_NC_CACHE = None


def _get_nc():
    global _NC_CACHE
    if _NC_CACHE is None:
        _NC_CACHE = build()
    return _NC_CACHE


def kernel(**inputs):
    from concourse.bass_utils import run_bass_kernel_spmd

    in_maps = _prep(inputs)
    nc = _get_nc()
    res = run_bass_kernel_spmd(nc, in_maps, list(range(NCORES)))
    out = np.concatenate([res.results[i]["out"] for i in range(NCORES)], axis=0)
    return np.ascontiguousarray(out, dtype=np.float32)


if __name__ == "__main__":
    print("smoke: building bass module...")
    nc = build()
    print("built ok:", sum(len(bb.instructions) for bb in nc.m.functions[0].blocks),
          "instructions")


# revision 81
# speedup vs baseline: 1.4480x; 1.0000x over previous
"""AttentionSubsample Trainium2 kernel (v2).

Full (unsharded) inputs in, full output out. Data-parallel over batch:
32 batches -> 8 NeuronCores x 4 batches each. Weights/biases replicated.

Key structure (vs v1 baseline at 275us; cost model now ~214us):
  - All non-PSUM tensors fp16 (same PE rate as bf16/f32r, 2x DVE modes,
    half the SBUF/DMA). Also ~7x more accurate than the bf16 v1.
  - Relative-position bias applied MULTIPLICATIVELY after exp:
    exp(s+b) = exp(s)*exp(b). exp(b) is precomputed on host, resident in
    SBUF (6.5MB, loaded once), applied with one tensor_tensor per group
    (mostly on the otherwise-idle Pool engine, DVE for first/last group)
    -- removes ~114us of f32 PSUM bias adds from DVE and the 26MB bias
    DMA stream entirely.
  - Softmax sums off the PE: e-chunk tree-accumulate on DVE (fp16, 2x,
    deps kept >=2 groups stale) + ONE ones-matmul per head on the
    accumulated [128,NQ] tile (PE 2560 cyc/batch instead of 25600).
  - k-channel BN bias dropped: softmax over n is invariant to per-q
    shifts, and (k+bk).q = k.q + bk.q[q] is exactly such a shift.
  - exp computed with a global -3 logit shift (ACT bias operand);
    softmax is exactly shift-invariant, and the shift keeps exp/sums
    inside fp16 range (logit max ~9 on this distribution).
  - PSUM: scores rotate through a dedicated 2-buf pool whose ONLY
    consumer is exp (ACT), so the sc->exp cadence never chains behind
    copies; projection/out-proj psums use a separate 2x1-bank pool
    (copy consumers); po/psm one bank each. Engines are in-order, so
    all ready work (o-matmuls of head h-1, proj units of batch b+1,
    out-proj of the previous pair) is emitted BEFORE each group's
    dependency-blocked scores, and the per-head tail (tree finish,
    ones-matmul, recip, normalize, hswish) is emitted one head late.
  - Batch-0 projections borrow the then-idle scores pool to double the
    warmup psum rotation depth.
"""

import sys

if "/opt/trn_rl_repo" not in sys.path:
    sys.path.insert(0, "/opt/trn_rl_repo")

import numpy as np

# --- problem constants (hardcoded, must match the grading reference) ---
B, N, C = 32, 1280, 256
H, KD, D = 8, 64, 128          # heads, key dim, value dim per head
NQ = 320                       # subsampled sequence length
OUT = 384
NCORES = 8
BPC = B // NCORES              # batches per core
EPS = 1e-5
NCH = N // 128                 # 10 n-chunks of 128
GRP = 2                        # chunks per score/exp group
NGR = NCH // GRP               # 5 groups per head
SHIFT = 3.0                    # global logit shift (exact for softmax)

_SUB_IDX = np.concatenate([
    (np.arange(32)[::2][:, None] * 32 + np.arange(32)[::2][None, :]).reshape(-1),
    1024 + (np.arange(16)[::2][:, None] * 16 + np.arange(16)[::2][None, :]).reshape(-1),
])  # [320] subsample row gather


def _prep(inputs):
    """Host-side: fold BN into weights, reorder channels, shard over cores."""
    f32 = np.float32
    f16 = np.float16
    x = np.asarray(inputs["x"], f32)
    g_kv, b_kv = np.asarray(inputs["g_kv"], f32), np.asarray(inputs["b_kv"], f32)
    rm_kv, rv_kv = np.asarray(inputs["rm_kv"], f32), np.asarray(inputs["rv_kv"], f32)
    g_q, b_q = np.asarray(inputs["g_q"], f32), np.asarray(inputs["b_q"], f32)
    rm_q, rv_q = np.asarray(inputs["rm_q"], f32), np.asarray(inputs["rv_q"], f32)
    g_p, b_p = np.asarray(inputs["g_p"], f32), np.asarray(inputs["b_p"], f32)
    rm_p, rv_p = np.asarray(inputs["rm_p"], f32), np.asarray(inputs["rv_p"], f32)
    W_kv = np.asarray(inputs["W_kv"], f32)
    W_q = np.asarray(inputs["W_q"], f32)
    W_p = np.asarray(inputs["W_p"], f32)
    attn_bias = np.asarray(inputs["attn_bias"], f32)
    bias_idxs = np.asarray(inputs["bias_idxs"])

    s_kv = g_kv / np.sqrt(rv_kv + EPS)
    Wkv_f = W_kv * s_kv[:, None]
    bkv_f = b_kv - rm_kv * s_kv
    kidx = np.concatenate([np.arange(h * 192, h * 192 + KD) for h in range(H)])
    vidx = np.concatenate([np.arange(h * 192 + KD, (h + 1) * 192) for h in range(H)])
    # k-channel bias dropped: softmax-invariant (see module docstring)
    wkt = np.ascontiguousarray(Wkv_f[kidx].T).reshape(2, 128, 512).astype(f16)
    wvt = np.ascontiguousarray(Wkv_f[vidx].T).reshape(2, 128, 1024).astype(f16)
    bvd = np.ascontiguousarray(bkv_f[vidx].reshape(8, 128).T)          # [128, H]

    scale = KD ** -0.5
    s_q = g_q / np.sqrt(rv_q + EPS)
    wqt = np.ascontiguousarray((W_q * (s_q * scale)[:, None]).T).reshape(
        2, 128, 512).astype(f16)
    bq = np.ascontiguousarray(((b_q - rm_q * s_q) * scale).reshape(4, 128).T)

    s_p = g_p / np.sqrt(rv_p + EPS)
    wpt = np.ascontiguousarray((W_p * s_p[:, None]).T / 6.0).reshape(
        8, 128, OUT).astype(f16)
    bp = np.ascontiguousarray(np.broadcast_to(b_p - rm_p * s_p, (128, OUT)))

    biasT = attn_bias[:, bias_idxs].transpose(0, 2, 1)                 # [H, N, NQ]
    eb = np.exp(biasT).reshape(H, NCH, 128, NQ).astype(f16)

    xs = x[:, _SUB_IDX, :]                                             # [B, NQ, C]
    in_maps = []
    for i in range(NCORES):
        sl = slice(i * BPC, (i + 1) * BPC)
        xt = np.ascontiguousarray(
            x[sl].transpose(0, 2, 1).reshape(BPC, 2, 128, N).astype(f16))
        xst = np.ascontiguousarray(
            xs[sl].transpose(0, 2, 1).reshape(BPC, 2, 128, NQ).astype(f16))
        in_maps.append({
            "xt": xt, "xst": xst,
            "wkt": wkt, "wvt": wvt, "wqt": wqt, "wpt": wpt,
            "bq": bq, "bv": bvd, "bp": bp,
            "eb": eb, "ones": np.ones((128, 128), f16),
        })
    return in_maps


def _body(tc, a, out_ap):
    import concourse.bass as bass  # noqa: F401
    import concourse.mybir as mybir
    from contextlib import ExitStack

    nc = tc.nc
    f32 = mybir.dt.float32
    f16 = mybir.dt.float16
    AF = mybir.ActivationFunctionType
    ALU = mybir.AluOpType

    with ExitStack() as ctx:
        ctx.enter_context(
            nc.allow_low_precision(reason="fp16 attention path is deliberate; "
                                   "verified against fp32 reference")
        )
        singles = ctx.enter_context(tc.tile_pool(name="singles", bufs=1))
        shiftc = singles.tile([128, 1], f32)
        nc.gpsimd.memset(shiftc, -SHIFT)
        bqs = singles.tile([128, 4], f32)
        nc.sync.dma_start(bqs, a["bq"])
        ones = singles.tile([128, 128], f16)
        nc.sync.dma_start(ones, a["ones"])
        wk = singles.tile([128, 2, 512], f16)
        nc.sync.dma_start(wk, a["wkt"].rearrange("c p j -> p c j"))
        wq = singles.tile([128, 2, 512], f16)
        nc.sync.dma_start(wq, a["wqt"].rearrange("c p j -> p c j"))
        wv = singles.tile([128, 2, 1024], f16)
        bvs = singles.tile([128, H], f32)
        wp = singles.tile([128, 8, OUT], f16)
        bps = singles.tile([128, OUT], f32)
        ebt = singles.tile([128, H, NCH, NQ], f16)

        xt_p = ctx.enter_context(tc.tile_pool(name="xt", bufs=2))
        xst_p = ctx.enter_context(tc.tile_pool(name="xst", bufs=2))
        kt_p = ctx.enter_context(tc.tile_pool(name="kt", bufs=2))
        v_p = ctx.enter_context(tc.tile_pool(name="v", bufs=2))
        qt_p = ctx.enter_context(tc.tile_pool(name="qt", bufs=2))
        e_p = ctx.enter_context(tc.tile_pool(name="e", bufs=20))
        acc_p = ctx.enter_context(tc.tile_pool(name="acc", bufs=3))
        rc_p = ctx.enter_context(tc.tile_pool(name="rc", bufs=3))
        ot_p = ctx.enter_context(tc.tile_pool(name="ot", bufs=1))
        hs_p = ctx.enter_context(tc.tile_pool(name="hs", bufs=2))
        ob_p = ctx.enter_context(tc.tile_pool(name="ob", bufs=2))
        orw_p = ctx.enter_context(tc.tile_pool(name="orw", bufs=2))
        # scores pool: consumed ONLY by exp (ACT) so the sc->exp rotation never
        # chains behind copies; unit pool: proj/outproj psums, copy consumers
        ps_work = ctx.enter_context(tc.tile_pool(name="ps_work", bufs=2, space="PSUM"))
        ps_unit = ctx.enter_context(tc.tile_pool(name="ps_unit", bufs=2, space="PSUM"))
        ps_o = ctx.enter_context(tc.tile_pool(name="ps_o", bufs=1, space="PSUM"))
        ps_sum = ctx.enter_context(tc.tile_pool(name="ps_sum", bufs=1, space="PSUM"))

        _wt_n = [0]

        def work_tile():
            _wt_n[0] += 1
            return ps_work.tile([128, GRP, 512], f32, tag="w", name=f"wt{_wt_n[0]}")

        def unit_tile(alt=False):
            _wt_n[0] += 1
            if alt:
                # batch-0 warmup: nothing competes for the scores pool yet,
                # so borrow it to double the proj-psum rotation depth
                return ps_work.tile([128, GRP, 512], f32, tag="w",
                                    name=f"ut{_wt_n[0]}")[:, 0, :]
            return ps_unit.tile([128, 512], f32, tag="u", name=f"ut{_wt_n[0]}")

        _cp_n = [0]

        def copy_engine():
            # ~60/40 ACT/DVE keeps the two psum-capable engines level
            _cp_n[0] += 1
            return nc.scalar.copy if _cp_n[0] % 3 < 1 else nc.vector.tensor_copy

        def emit_eb_dma():
            for h in range(H):
                nc.sync.dma_start(
                    ebt[:, h, :, :],
                    a["eb"][h].rearrange("c p q -> p c q"),
                )

        # --- projection emission for one batch, as a list of closures ---
        def proj_units(b, xt, xst, kt, vt, qt, first_batch=False):
            units = []

            def kt_unit(pr, ns, alt=False):
                def run():
                    n0 = ns * 512
                    nsz = min(512, N - n0)
                    ps = unit_tile(alt)
                    for cc in range(2):
                        nc.tensor.matmul(
                            ps[:, :nsz],
                            lhsT=wk[:, cc, pr * 128:(pr + 1) * 128],
                            rhs=xt[:, cc, n0:n0 + nsz],
                            start=(cc == 0), stop=(cc == 1),
                        )
                    copy_engine()(kt[:, pr, n0:n0 + nsz], ps[:, :nsz])
                return run

            def v_unit(cn, hf, alt=False):
                def run():
                    ps = unit_tile(alt)
                    for cc in range(2):
                        nc.tensor.matmul(
                            ps,
                            lhsT=xt[:, cc, cn * 128:(cn + 1) * 128],
                            rhs=wv[:, cc, hf * 512:(hf + 1) * 512],
                            start=(cc == 0), stop=(cc == 1),
                        )
                    copy_engine()(vt[:, cn, hf * 512:(hf + 1) * 512], ps)
                return run

            def q_unit(pr, alt=False):
                def run():
                    ps = unit_tile(alt)
                    for cc in range(2):
                        nc.tensor.matmul(
                            ps[:, :NQ],
                            lhsT=wq[:, cc, pr * 128:(pr + 1) * 128],
                            rhs=xst[:, cc, :],
                            start=(cc == 0), stop=(cc == 1),
                        )
                    nc.scalar.activation(
                        qt[:, pr, :], ps[:, :NQ], AF.Identity,
                        bias=bqs[:, pr:pr + 1],
                    )
                return run

            i = [0]

            def alt():
                i[0] += 1
                return first_batch and i[0] % 2 == 0

            for pr in range(4):
                for ns in range(3):
                    units.append(kt_unit(pr, ns, alt()))
            for cn in range(NCH):
                for hf in range(2):
                    units.append(v_unit(cn, hf, alt()))
            for pr in range(4):
                units.append(q_unit(pr, alt()))
            return units

        # --- per-batch state for the software pipeline ---
        def head_attention(b, kt, vt, qt, ot, units_next):
            """Head loop with one-head o-matmul lag + proj interleaving."""
            nunits = len(units_next)
            consumed = [0]

            def consume(k):
                while consumed[0] < min(nunits, k):
                    units_next[consumed[0]]()
                    consumed[0] += 1

            e_tiles = [[None] * NGR for _ in range(H)]
            po_tiles = [None] * H
            orw_tiles = [None] * H
            rc_tiles = [None] * H
            psm_tiles = [None] * H

            def emit_o(h, g):
                po = po_tiles[h]
                for j in range(GRP):
                    c = GRP * g + j
                    nc.tensor.matmul(
                        po,
                        lhsT=vt[:, c, h * 128:(h + 1) * 128],
                        rhs=e_tiles[h][g][:, j, :],
                        start=(c == 0), stop=(c == NCH - 1),
                    )

            def emit_oraw(h):
                # po -> sbuf on ACT (frees the po bank fast; runs while DVE
                # finishes the sum tree)
                oraw = orw_p.tile([128, NQ], f16, tag="orw", name=f"orw{b}_{h}")
                nc.scalar.copy(oraw, po_tiles[h])
                orw_tiles[h] = oraw

            def emit_norm(h):
                # normalize multiply in DVE 2x mode off-PSUM
                oh = ot[:, h, :]
                nc.vector.tensor_tensor(oh, orw_tiles[h], rc_tiles[h], ALU.mult)
                nc.vector.tensor_scalar_add(oh, oh, bvs[:, h:h + 1])

            def emit_hswish(h0, t):
                # t = o*relu6(o+3), 1/6 folded into wp; two heads at once
                o2 = ot[:, h0:h0 + 2, :]
                t2 = t[:, h0:h0 + 2, :]
                nc.vector.tensor_scalar(t2, o2, 3.0, 6.0, ALU.add, ALU.min)
                nc.vector.tensor_scalar(t2, t2, 0.0, None, ALU.max)
                nc.vector.tensor_tensor(t2, t2, o2, ALU.mult)

            # hswish output doubles as out-proj lhsT for a batch PAIR:
            # q merged to 640 = 5x128 exact (vs 2.5x128 per batch)
            if b % 2 == 0:
                t_pair = hs_p.tile([128, H, 2, NQ], f16, tag="t", name=f"t_{b}")
                ts[b] = t_pair
            else:
                t_pair = ts[b - 1]
            t = t_pair[:, :, b % 2, :]
            acc_tiles = [None] * H

            def head_tail(h):
                # everything past the in-loop sum-tree for head h, emitted one
                # head late so no engine waits on the exp->mult->add chain
                emit_oraw(h)
                acc = acc_tiles[h]
                nc.vector.tensor_tensor(acc, acc, e_tiles[h][NGR - 2], ALU.add)
                nc.vector.tensor_tensor(acc, acc, e_tiles[h][NGR - 1], ALU.add)
                psm = ps_sum.tile([128, NQ], f32, tag="psm", name=f"psm{b}_{h}")
                nc.tensor.matmul(psm, lhsT=ones, rhs=acc[:, 0, :],
                                 start=True, stop=False)
                nc.tensor.matmul(psm, lhsT=ones, rhs=acc[:, 1, :],
                                 start=False, stop=True)
                rc_tiles[h] = rc_p.tile([128, NQ], f16, tag="rc", name=f"rc{b}_{h}")
                nc.vector.reciprocal(rc_tiles[h], psm)
                emit_norm(h)
                if h % 2 == 1:
                    emit_hswish(h - 1, t)

            for h in range(H):
                pr, p0 = h // 2, 64 * (h % 2)
                po_tiles[h] = ps_o.tile([128, NQ], f32, tag="po", name=f"po{b}_{h}")
                acc = acc_p.tile([128, GRP, NQ], f16, tag="acc", name=f"acc{b}_{h}")
                acc_tiles[h] = acc
                for g in range(NGR):
                    # READY work first: in-order engines, so anything emitted
                    # after the (dependency-blocked) scores would stall behind
                    # them. o-matmuls of head h-1 and proj units have ancient
                    # deps and fill the PE while exp/mult of g-2..g-1 complete.
                    if h > 0:
                        emit_o(h - 1, g)
                        consume((nunits * (h * NGR + g + 1)) // (H * NGR))
                    else:
                        consume(2 * (g + 1))
                    sg = work_tile()
                    for j in range(GRP):
                        c = GRP * g + j
                        nc.tensor.matmul(
                            sg[:, j, :NQ],
                            lhsT=kt[p0:p0 + 64, pr, c * 128:(c + 1) * 128],
                            rhs=qt[p0:p0 + 64, pr, :],
                            start=True, stop=True,
                        )
                    e = e_p.tile([128, GRP, NQ], f16, tag="e", name=f"e{b}_{h}_{g}")
                    e_tiles[h][g] = e
                    nc.scalar.activation(e, sg[:, :, :NQ], AF.Exp, bias=shiftc[:, 0:1])
                    # bias-mult is SBUF-only: Pool takes the middle groups,
                    # DVE the first/last
                    eng = nc.vector if g in (0, NGR - 1) else nc.gpsimd
                    eng.tensor_tensor(
                        e, e, ebt[:, h, GRP * g:GRP * (g + 1), :], ALU.mult)
                    # sums tree on DVE, two groups STALE so the add never
                    # waits on a Pool mult in flight (e0 is summed by PE
                    # ones-matmuls in head_tail instead)
                    if g == 3:
                        nc.vector.tensor_tensor(
                            acc, e_tiles[h][0], e_tiles[h][1], ALU.add)
                    elif g == 4:
                        nc.vector.tensor_tensor(
                            acc, acc, e_tiles[h][2], ALU.add)
                if h > 0:
                    head_tail(h - 1)
            # drain last head
            for g in range(NGR):
                emit_o(H - 1, g)
            head_tail(H - 1)
            consume(nunits)

        # --- main loop over batches ---
        kts, vts, qts, ts = {}, {}, {}, {}

        def alloc_and_dma(b):
            xt = xt_p.tile([128, 2, N], f16)
            for ns in range(3):
                n0 = ns * 512
                nsz = min(512, N - n0)
                nc.sync.dma_start(
                    xt[:, :, n0:n0 + nsz],
                    a["xt"][b, :, :, n0:n0 + nsz].rearrange("c p n -> p c n"),
                )
            xst = xst_p.tile([128, 2, NQ], f16)
            nc.sync.dma_start(xst, a["xst"][b].rearrange("c p n -> p c n"))
            kts[b] = kt_p.tile([128, 4, N], f16, tag="kt", name=f"kt{b}")
            vts[b] = v_p.tile([128, NCH, 1024], f16, tag="vt", name=f"vt{b}")
            qts[b] = qt_p.tile([128, 4, NQ], f16, tag="qt", name=f"qt{b}")
            return xt, xst

        def outproj_units(b):
            # output projection for batch pair (b-1, b), q merged to 640;
            # returned as closures consumed inside the NEXT batch's head loop
            out_flat = out_ap.rearrange("b q o -> (b q) o")
            t_pair = ts[b - 1]

            def qc_unit(qc):
                def run():
                    r0 = (b - 1) * NQ + qc * 128
                    ps = unit_tile()
                    for dc in range(8):
                        nc.tensor.matmul(
                            ps[:, :OUT],
                            lhsT=t_pair[:, dc, :, :].rearrange(
                                "p bb q -> p (bb q)")[:, qc * 128:(qc + 1) * 128],
                            rhs=wp[:, dc, :],
                            start=(dc == 0), stop=(dc == 7),
                        )
                    ob = ob_p.tile([128, OUT], f32)
                    nc.vector.tensor_tensor(ob, ps[:, :OUT], bps, ALU.add)
                    nc.sync.dma_start(out_flat[r0:r0 + 128, :], ob)
                return run

            return [qc_unit(qc) for qc in range(5)]

        xts = {}
        xts[0] = alloc_and_dma(0)
        # weights needed later in batch 0: emit DMA behind the first x tiles
        nc.sync.dma_start(wv, a["wvt"].rearrange("c p j -> p c j"))
        nc.sync.dma_start(bvs, a["bv"])
        emit_eb_dma()
        nc.sync.dma_start(wp, a["wpt"].rearrange("c p j -> p c j"))
        nc.sync.dma_start(bps, a["bp"])

        units0 = proj_units(0, *xts[0], kts[0], vts[0], qts[0], first_batch=True)
        for u in units0:
            u()
        carry_out = []
        for b in range(BPC):
            if b + 1 < BPC:
                xts[b + 1] = alloc_and_dma(b + 1)
                units_next = proj_units(
                    b + 1, *xts[b + 1], kts[b + 1], vts[b + 1], qts[b + 1])
            else:
                units_next = []
            units = list(units_next)
            # weave last pair's out-proj into the early-middle of this batch
            for i, u in enumerate(carry_out):
                pos = min(len(units), 6 + 7 * i)
                units.insert(pos, u)
            head_attention(b, kts[b], vts[b], qts[b],
                           ot_p.tile([128, H, NQ], f16, tag="ot", name=f"ot{b}"),
                           units)
            carry_out = outproj_units(b) if b % 2 == 1 else []
        for u in carry_out:
            u()


def build():
    import concourse.mybir as mybir
    import concourse.tile as tile
    from concourse import bacc

    nc = bacc.Bacc("TRN2", target_bir_lowering=False, debug=False)
    f32, f16 = mybir.dt.float32, mybir.dt.float16
    a = {}

    def din(name, shape, dt=f32):
        a[name] = nc.dram_tensor(name, shape, dt, kind="ExternalInput").ap()

    din("xt", [BPC, 2, 128, N], f16)
    din("xst", [BPC, 2, 128, NQ], f16)
    din("wkt", [2, 128, 512], f16)
    din("wvt", [2, 128, 1024], f16)
    din("wqt", [2, 128, 512], f16)
    din("wpt", [8, 128, OUT], f16)
    din("bq", [128, 4])
    din("bv", [128, H])
    din("bp", [128, OUT])
    din("eb", [H, NCH, 128, NQ], f16)
    din("ones", [128, 128], f16)
    out_ap = nc.dram_tensor("out", [BPC, NQ, OUT], f32, kind="ExternalOutput").ap()

    with tile.TileContext(nc) as tc:
        _body(tc, a, out_ap)
    nc.compile()
    return nc


_NC_CACHE = None


def _get_nc():
    global _NC_CACHE
    if _NC_CACHE is None:
        _NC_CACHE = build()
    return _NC_CACHE


def kernel(**inputs):
    from concourse.bass_utils import run_bass_kernel_spmd

    in_maps = _prep(inputs)
    nc = _get_nc()
    res = run_bass_kernel_spmd(nc, in_maps, list(range(NCORES)))
    out = np.concatenate([res.results[i]["out"] for i in range(NCORES)], axis=0)
    return np.ascontiguousarray(out, dtype=np.float32)


if __name__ == "__main__":
    print("smoke: building bass module...")
    nc = build()
    print("built ok:", sum(len(bb.instructions) for bb in nc.m.functions[0].blocks),
          "instructions")
